# revision 28
# baseline (speedup 1.0000x reference)
"""Trainium2 Bass kernel for a 2-layer GAT occupancy predictor (B=1).

Reference math:
  pts = concat(pos, pos_non_manifold) -> [K=6000, 3]
  mask[i,j] = ||pts_i - pts_j||^2 < 0.05^2          (dense radius graph)
  layer l:  h = x @ Wl                              [K, 4*64]
            e[i,j,h] = leaky02(ed[i,h] + es[j,h])   es/ed = <h, a_src/dst>
            alpha = softmax_j(e masked)
            x' = relu(alpha @ h + b)
  logits = (x2 @ fc_w + fc_b)[M:] reshaped to [1, 2, 3000]

Distribution (8 NeuronCores): nodes are Morton-sorted; core c owns the 768
destinations [768c, 768(c+1)) of the padded 6144-node graph.

Slot structure per core (T = 6 + XT slots of 128 sources each):
  slots 0-5 : own Morton blocks in identity order.  Each only processes
              the dst WINDOW [128s-128, 128s+256) -- Morton locality puts
              nearly all of a block's edges there (~58% of the dense
              volume).  Out-of-window edges are re-covered by the extra
              slots below.
  slots 6+  : "extra" tiles = remote sources (other cores) plus own
              "fixup" nodes that have any out-of-window edge.  These
              process the full 768-dst range; a host-built, index-only
              wmn mask (-60000 on the in-window range of each fixup
              lane) removes the pairs already covered by the own slots.

Everything 16-bit on the hot path (fp16), f32 accumulation in PSUM.
Aggregation PSUM is k-major [128, 6, H, 128] so each 128-dst chunk k is
one 2KB PSUM bank; the first slot touching chunk k issues start=True on
its head-0 matmul (clearing the whole bank), later heads/slots ride
has_written=0 overwrite/accumulate semantics.

Between layers: x1^T assembled by partition-moving DMAs; h2 = x1 @ W2
(+es ride-along) computed per-owner; only the E exported boundary rows
(the rows some peer actually consumes) are AllGathered as fp16 rows
[h0|1|h1|1|h2|1|h3|1|es4].  Own slots read h2 straight from the
resident hg_sb buffer and overlap the AllGather; extra slots assemble
their source tiles with two bounds-checked indirect gathers (own fixup
lanes from local hg_dram, remote lanes from the AllGather output).
Masks bounce through DRAM between layers.
"""

import sys

sys.path.insert(0, "/opt/trn_rl_repo")

from contextlib import ExitStack

import ml_dtypes
import numpy as np

import concourse.bacc as bacc
import concourse.bass as bass
import concourse.mybir as mybir
import concourse.tile as tile
from concourse.bass_utils import run_bass_kernel_spmd

F32 = mybir.dt.float32
F16 = mybir.dt.float16
I32 = mybir.dt.int32
AF = mybir.ActivationFunctionType
OP = mybir.AluOpType
AX = mybir.AxisListType

N_CORES = 8
N = 3000
M = 3000
K = N + M          # real nodes
KP = 6144          # padded nodes
IC = KP // N_CORES # 768 destinations per core
NOWN = IC // 128   # 6 own slots
H = 4              # heads
C = 64             # channels per head
HC = H * C         # 256
HCE = HC + H       # 260: h columns + es columns (layer-2 ride-along)
ROWW = H * (C + 1) + H  # 264: AG row [h0|1|h1|1|h2|1|h3|1|es4]
R2 = float(np.float32(0.05) * np.float32(0.05))
PAD_COORD = -1.0
PAD_NODE = KP - 1
MASK_EPS = 1e-5    # host activity-test margin (superset of device mask)
MNEG = -60000.0    # masked-score offset; *0.2 then exp -> 0 in fp16
GA = 384           # d2/mask column chunk (PSUM bank budget)
W_LO, W_HI = 128, 256   # own-slot dst window [128s-W_LO, 128s+W_HI)
BIGIDX = 1 << 20   # skip sentinel for bounds-checked indirect gathers


def _windows(T):
    win = [(max(0, 128 * s - W_LO), min(IC, 128 * s + W_HI))
           for s in range(NOWN)]
    win += [(0, IC)] * (T - NOWN)
    return win


def build(nslot, nexp, n_cores=N_CORES, fake_ag=False):
    nc = bacc.Bacc("TRN2", target_bir_lowering=False, debug=False,
                   num_devices=n_cores)
    T = nslot
    E = nexp
    XT = T - NOWN
    WIN = _windows(T)
    # L1 runs the full-range extra tiles FIRST so the windowed own slots
    # finish each dst chunk k as early as possible -- the finalize for
    # k-group A (chunks 0-2) then overlaps the last own slots.  L2 keeps
    # own slots first (they hide the AllGather) and extras last.
    ORD = {1: list(range(NOWN, T)) + list(range(NOWN)),
           2: list(range(T))}
    first_pos, last_pos = {}, {}
    for layer in (1, 2):
        fp, lp = {}, {}
        for pos, s in enumerate(ORD[layer]):
            lo, hi = WIN[s]
            for k in range(lo // 128, hi // 128):
                if k not in fp:
                    fp[k] = pos
                lp[k] = pos
        first_pos[layer], last_pos[layer] = fp, lp

    # ---- kernel I/O (identical program on every core) ----
    sel5_d = nc.dram_tensor("sel5", [5, T * 128], F32, kind="ExternalInput")
    # own5ge: cols 0:768 = [2p; -1; R2-sq] (g = R2-d2), cols 768:772 = es1
    own5ge_d = nc.dram_tensor("own5ge", [5, IC + H], F32,
                              kind="ExternalInput")
    own3_d = nc.dram_tensor("own3", [3, IC], F32, kind="ExternalInput")
    agidx_d = nc.dram_tensor("agidx", [128, T], I32, kind="ExternalInput")
    locidx_d = nc.dram_tensor("locidx", [128, XT], I32,
                              kind="ExternalInput")
    scatidx_d = nc.dram_tensor("scatidx", [128, NOWN], I32,
                               kind="ExternalInput")
    wmn_d = nc.dram_tensor("wmn", [128, XT, IC], F16, kind="ExternalInput")
    w1p_d = nc.dram_tensor("w1p", [3, HC], F32, kind="ExternalInput")
    w1d_d = nc.dram_tensor("w1d", [3, H], F32, kind="ExternalInput")
    w2p_d = nc.dram_tensor("w2p", [HC, HCE], F16, kind="ExternalInput")
    admw2_d = nc.dram_tensor("admw2", [HC, H], F16, kind="ExternalInput")
    b1t_d = nc.dram_tensor("b1t", [C, H], F32, kind="ExternalInput")
    b2t_d = nc.dram_tensor("b2t", [C, H], F32, kind="ExternalInput")
    fcw_d = nc.dram_tensor("fcw", [C, H, 2], F16, kind="ExternalInput")
    fcb_d = nc.dram_tensor("fcb", [128, 2], F32, kind="ExternalInput")

    out_d = nc.dram_tensor("out", [IC, 2], F32, kind="ExternalOutput")

    with tile.TileContext(nc) as tc, ExitStack() as st:
        dram = st.enter_context(tc.tile_pool(name="dram", bufs=1,
                                             space="DRAM"))
        mn_dram = dram.tile([T, 128, IC], F16)
        hg_dram = dram.tile([IC, ROWW], F16)
        hgx_dram = dram.tile([E, ROWW], F16)
        ag_out = dram.tile([n_cores * E, ROWW], F16,
                           addr_space=("Local" if fake_ag else "Shared"))

        const = st.enter_context(tc.tile_pool(name="const", bufs=1))
        sel5_sb = const.tile([5, T * 128], F32)
        own5ge_sb = const.tile([5, IC + H], F32)
        own3_sb = const.tile([3, IC], F32)
        agidx_sb = const.tile([128, T], I32)
        locidx_sb = const.tile([128, XT], I32)
        scatidx_sb = const.tile([128, NOWN], I32)
        wmn_sb = const.tile([128, XT, IC], F16)
        w1p_sb = const.tile([3, HC], F32)
        w1d_sb = const.tile([3, H], F32)
        w2p_sb = const.tile([128, 2, HCE], F16)
        admw2_sb = const.tile([128, 2, H], F16)
        b1t_sb = const.tile([C, H], F32)
        b2t_sb = const.tile([C, H], F32)
        fcw_sb = const.tile([C, H, 2], F16)
        fcb_sb = const.tile([128, 2], F32)

        nc.sync.dma_start(out=own3_sb[:, :], in_=own3_d[:, :])
        nc.sync.dma_start(out=w1d_sb[:, :], in_=w1d_d[:, :])
        nc.sync.dma_start(out=sel5_sb[:, :], in_=sel5_d[:, :])
        nc.sync.dma_start(out=own5ge_sb[:, :], in_=own5ge_d[:, :])
        nc.sync.dma_start(out=w1p_sb[:, :], in_=w1p_d[:, :])
        nc.sync.dma_start(out=agidx_sb[:, :], in_=agidx_d[:, :])
        nc.sync.dma_start(out=locidx_sb[:, :], in_=locidx_d[:, :])
        nc.sync.dma_start(out=scatidx_sb[:, :], in_=scatidx_d[:, :])
        nc.sync.dma_start(out=wmn_sb[:, :, :], in_=wmn_d[:, :, :])
        nc.sync.dma_start(out=w2p_sb[:, :, :],
                          in_=w2p_d.rearrange("(s p) c -> p s c", p=128))
        nc.sync.dma_start(out=admw2_sb[:, :, :],
                          in_=admw2_d.rearrange("(s p) h -> p s h", p=128))
        nc.sync.dma_start(out=b1t_sb[:, :], in_=b1t_d[:, :])
        nc.sync.dma_start(out=b2t_sb[:, :], in_=b2t_d[:, :])
        nc.sync.dma_start(out=fcw_sb[:, :, :], in_=fcw_d[:, :, :])
        nc.sync.dma_start(out=fcb_sb[:, :], in_=fcb_d[:, :])

        big = st.enter_context(tc.tile_pool(name="big", bufs=1))
        # layer-1 source features, AG-row layout [h0|1|h1|1|h2|1|h3|1|es4]
        hsrc = big.tile([128, T, ROWW], F16)
        es4f = big.tile([128, T, H], F32)
        ed_b = big.tile([128, H, IC], F16)
        edt_sb = big.tile([H, IC], F16)
        edt_row = big.tile([1, H, IC], F16)
        x1T = big.tile([128, 2, IC], F16)
        x2T = big.tile([128, 2, IC], F16)
        hg_sb = big.tile([128, IC // 128, ROWW], F16)
        xr = big.tile([C, H, IC], F16)
        # k-major finalize staging: flat order (k, h, d)
        den_sb = big.tile([128, NOWN, H, 128], F32)
        dinv_sb = big.tile([128, NOWN, H, 128], F32)
        dinv_row = big.tile([1, NOWN, H, 128], F32)
        dinv_b = big.tile([128, NOWN, H, 128], F32)
        xc = big.tile([C, NOWN, H, 128], F16)
        logit_sb = big.tile([128, IC // 128, 2], F32)

        h65 = hsrc[:, :, 0:H * (C + 1)].rearrange("p t (h x) -> p t h x", h=H)
        nc.vector.memset(h65[:, :, :, C:C + 1], 1.0)
        g65 = hg_sb[:, :, 0:H * (C + 1)].rearrange("p q (h x) -> p q h x",
                                                   h=H)
        nc.vector.memset(g65[:, :, :, C:C + 1], 1.0)

        for layer in (1, 2):
            # ---- prep: edt rows + partition-broadcast to ed_b ----
            with tc.tile_pool(name=f"prep{layer}", bufs=1,
                              space="PSUM") as prep_ps:
                edt_ps = prep_ps.tile([H, IC], F32, tag="edt")
                # 384-col chunks align with the finalize k-groups, so
                # layer-2 edt chunk A only waits on x1T group A
                for lo, sz in ((0, 384), (384, 384)):
                    sl = slice(lo, lo + sz)
                    if layer == 1:
                        nc.tensor.matmul(edt_ps[:, sl], w1d_sb[:, :],
                                         own3_sb[:, sl],
                                         start=True, stop=True)
                    else:
                        for s2 in range(2):
                            nc.tensor.matmul(edt_ps[:, sl],
                                             admw2_sb[:, s2, :],
                                             x1T[:, s2, sl],
                                             start=(s2 == 0), stop=(s2 == 1))
                nc.vector.tensor_scalar_add(edt_sb[:, :], edt_ps[:, :],
                                            0.0)
            # one partition-moving DMA + one broadcast for all 4 heads
            nc.sync.dma_start(out=edt_row[0:1, :, :], in_=edt_sb[:, :])
            nc.gpsimd.partition_broadcast(
                ed_b[:, :, :].rearrange("p h d -> p (h d)"),
                edt_row[0:1, :, :].rearrange("p h d -> p (h d)"))

            # ---- slot loop ----
            with tc.tile_pool(name=f"agg_ps{layer}", bufs=1,
                              space="PSUM") as agg_pool:
                # k-major: chunk k of 128 dsts = one 2KB PSUM bank
                agg_ps = agg_pool.tile([128, NOWN, H, 128], F32, tag="agg",
                                       name=f"agg_{layer}")
                with tc.tile_pool(name=f"jl{layer}", bufs=4) as jl, \
                     tc.tile_pool(name=f"mnp{layer}", bufs=3) as mnp, \
                     tc.tile_pool(name=f"h_ps{layer}", bufs=1,
                                  space="PSUM") as h_psp:
                    for pos, s in enumerate(ORD[layer]):
                        lo, hi = WIN[s]
                        Ws = hi - lo
                        mn = mnp.tile([128, IC], F16, tag="mn",
                                      name=f"mn_{layer}_{s}")
                        if layer == 1:
                            h_ps = h_psp.tile([128, HC], F32, tag="h",
                                              name=f"h_ps_{s}")
                            nc.tensor.matmul(
                                h_ps[:, :],
                                sel5_sb[0:3, s * 128:(s + 1) * 128],
                                w1p_sb[:, :], start=True, stop=True)
                            nc.scalar.copy(
                                h65[:, s, :, 0:C],
                                h_ps[:, :].rearrange("p (h c) -> p h c",
                                                     h=H))
                            g_ps = h_psp.tile([128, GA + H], F32, tag="g",
                                              name=f"g_ps_{s}")
                            if s < NOWN:
                                # one windowed g chunk + es ride columns
                                nc.tensor.matmul(
                                    g_ps[:, 0:Ws],
                                    sel5_sb[:, s * 128:(s + 1) * 128],
                                    own5ge_sb[:, lo:hi],
                                    start=True, stop=True)
                                nc.tensor.matmul(
                                    g_ps[:, GA:GA + H],
                                    sel5_sb[:, s * 128:(s + 1) * 128],
                                    own5ge_sb[:, IC:IC + H],
                                    start=True, stop=True)
                                nc.vector.tensor_scalar(
                                    mn[:, 0:Ws], g_ps[:, 0:Ws], 0.0, MNEG,
                                    OP.is_lt, OP.mult)
                                nc.vector.tensor_scalar_add(
                                    es4f[:, s, :], g_ps[:, GA:GA + H], 0.0)
                            else:
                                # full-range g in two chunks (+es), then
                                # the index-only window mask removes the
                                # pairs the own slots already covered.
                                nc.tensor.matmul(
                                    g_ps[:, 0:GA],
                                    sel5_sb[:, s * 128:(s + 1) * 128],
                                    own5ge_sb[:, 0:GA],
                                    start=True, stop=True)
                                nc.vector.tensor_scalar(
                                    mn[:, 0:GA], g_ps[:, 0:GA], 0.0, MNEG,
                                    OP.is_lt, OP.mult)
                                nc.tensor.matmul(
                                    g_ps[:, :],
                                    sel5_sb[:, s * 128:(s + 1) * 128],
                                    own5ge_sb[:, GA:IC + H],
                                    start=True, stop=True)
                                nc.vector.tensor_scalar(
                                    mn[:, GA:IC], g_ps[:, 0:IC - GA], 0.0,
                                    MNEG, OP.is_lt, OP.mult)
                                nc.vector.tensor_scalar_add(
                                    es4f[:, s, :],
                                    g_ps[:, IC - GA:IC - GA + H], 0.0)
                                nc.vector.tensor_tensor(
                                    mn[:, :], mn[:, :],
                                    wmn_sb[:, s - NOWN, :], OP.min)
                            nc.sync.dma_start(out=mn_dram[s, :, lo:hi],
                                              in_=mn[:, 0:Ws])
                            src = hsrc[:, s, :]
                            es_ap = es4f[:, s, :]
                        else:
                            if s < NOWN:
                                # own-node slots: h2 rows resident in hg_sb
                                src = hg_sb[:, s, :]
                            else:
                                src = jl.tile([128, ROWW], F16, tag="hg",
                                              name=f"hg_{s}")
                                # fixup lanes from local hg rows (early,
                                # no AllGather dependency)...
                                nc.gpsimd.indirect_dma_start(
                                    out=src[:, :], out_offset=None,
                                    in_=hg_dram[:, :],
                                    in_offset=bass.IndirectOffsetOnAxis(
                                        ap=locidx_sb[:, s - NOWN:s - NOWN + 1],
                                        axis=0),
                                    bounds_check=IC - 1, oob_is_err=False)
                                # ...remote lanes from the AllGather
                                nc.gpsimd.indirect_dma_start(
                                    out=src[:, :], out_offset=None,
                                    in_=ag_out[:, :],
                                    in_offset=bass.IndirectOffsetOnAxis(
                                        ap=agidx_sb[:, s:s + 1], axis=0),
                                    bounds_check=n_cores * E - 1,
                                    oob_is_err=False)
                            nc.sync.dma_start(out=mn[:, 0:Ws],
                                              in_=mn_dram[s, :, lo:hi])
                            esg = jl.tile([128, H], F32, tag="esg",
                                          name=f"esg_{s}")
                            nc.vector.tensor_scalar_add(
                                esg[:, :],
                                src[:, H * (C + 1):ROWW], 0.0)
                            es_ap = esg[:, :]

                        # scores: L = leaky02(ed + es + mn); A = exp(L).
                        # u4 = ed + mn in ONE 2x TT via a stride-0 head
                        # broadcast of mn.  Then heads 0-1 get es+leaky via
                        # ACT Prelu (bias=es); heads 2-3 via 4x TS es-adds
                        # and a TS/TT leaky (STT only has a 1x uop).
                        L4 = jl.tile([128, H, IC], F16, tag="L4",
                                     name=f"L4_{layer}_{s}")
                        u4 = jl.tile([128, H, IC], F16, tag="u4",
                                     name=f"u4_{layer}_{s}")
                        ub, mb = bass.broadcast_tensor_aps(
                            ed_b[:, :, lo:hi],
                            mn[:, 0:Ws].rearrange("p (o d) -> p o d", o=1))
                        nc.vector.tensor_tensor(u4[:, :, 0:Ws], ub, mb,
                                                OP.add)
                        for h in range(2):
                            nc.scalar.activation(
                                L4[:, h, 0:Ws], u4[:, h, 0:Ws], AF.Prelu,
                                bias=es_ap[:, h:h + 1],
                                scale=1.0, alpha=0.2)
                        v2 = jl.tile([128, 2, IC], F16, tag="v2",
                                     name=f"v2_{layer}_{s}")
                        for h in range(2, H):
                            nc.vector.tensor_scalar_add(
                                v2[:, h - 2, 0:Ws], u4[:, h, 0:Ws],
                                es_ap[:, h:h + 1])
                        t2 = jl.tile([128, 2, IC], F16, tag="t2",
                                     name=f"t2_{layer}_{s}")
                        nc.vector.tensor_scalar_mul(t2[:, :, 0:Ws],
                                                    v2[:, :, 0:Ws], 0.2)
                        nc.vector.tensor_tensor(L4[:, 2:4, 0:Ws],
                                                v2[:, :, 0:Ws],
                                                t2[:, :, 0:Ws], OP.max)
                        A4 = jl.tile([128, H, IC], F16, tag="A4",
                                     name=f"A4_{layer}_{s}")
                        nc.scalar.activation(A4[:, :, 0:Ws],
                                             L4[:, :, 0:Ws], AF.Exp)

                        # transposed aggregation: [h|ones] stationary,
                        # one 128-dst chunk (=one PSUM bank) per matmul.
                        # start=True (bank clear) only on the head-0
                        # matmul of the first slot touching chunk k.
                        for h in range(H):
                            for k in range(lo // 128, hi // 128):
                                nc.tensor.matmul(
                                    agg_ps[0:C + 1, k, h, :],
                                    src[:, h * (C + 1):(h + 1) * (C + 1)],
                                    A4[:, h, k * 128 - lo:k * 128 - lo + 128],
                                    start=(pos == first_pos[layer][k]
                                           and h == 0),
                                    stop=(pos == last_pos[layer][k]))

                # ---- finalize: x^T = relu(num*dinv + b) ----
                bt_sb = b1t_sb if layer == 1 else b2t_sb
                xT = x1T if layer == 1 else x2T
                # k-group split: group A (chunks 0-2) stops accumulating
                # before group B in layer 1 (extras-first order), so its
                # whole den->dinv->broadcast->mult chain overlaps the
                # remaining own slots.
                KH = [slice(0, NOWN // 2), slice(NOWN // 2, NOWN)]
                fin_st = ExitStack()
                if layer == 2:
                    fc_ps_pool = fin_st.enter_context(
                        tc.tile_pool(name="fc", bufs=1, space="PSUM"))
                    logit_ps = fc_ps_pool.tile([128, IC // 128, 2], F32,
                                               tag="lg")
                for g2 in range(2):
                    ks = KH[g2]
                    # 1/den as exp(-ln(den)): two ACT table ops straight
                    # from PSUM beat the DVE iterative divide ~5x.
                    nc.scalar.activation(
                        den_sb[C:C + 1, ks, :, :],
                        agg_ps[C:C + 1, ks, :, :],
                        AF.Ln)
                    nc.scalar.activation(
                        dinv_sb[C:C + 1, ks, :, :],
                        den_sb[C:C + 1, ks, :, :],
                        AF.Exp, scale=-1.0)
                    nc.sync.dma_start(
                        out=dinv_row[0:1, ks, :, :],
                        in_=dinv_sb[C:C + 1, ks, :, :])
                    nc.gpsimd.partition_broadcast(
                        dinv_b[0:C, ks, :, :].rearrange(
                            "p k h d -> p (k h d)"),
                        dinv_row[0:1, ks, :, :].rearrange(
                            "p k h d -> p (k h d)"))
                    nc.vector.tensor_tensor(
                        xc[:, ks, :, :], agg_ps[0:C, ks, :, :],
                        dinv_b[0:C, ks, :, :], OP.mult)
                    gcol = slice(g2 * (IC // 2), (g2 + 1) * (IC // 2))
                    for h in range(H):
                        nc.vector.tensor_scalar(
                            xr[:, h, gcol].rearrange("p (k d) -> p k d",
                                                     k=NOWN // 2),
                            xc[:, ks, h, :], bt_sb[:, h:h + 1],
                            0.0, OP.add, OP.max)
                        if layer == 1:
                            po = (h % 2) * C
                            nc.sync.dma_start(
                                out=xT[po:po + C, h // 2, gcol],
                                in_=xr[0:C, h, gcol])
                        if layer == 2:
                            for oc in range(g2 * (NOWN // 2),
                                            (g2 + 1) * (NOWN // 2)):
                                nc.tensor.matmul(
                                    logit_ps[:, oc, :],
                                    xr[0:C, h, oc * 128:(oc + 1) * 128],
                                    fcw_sb[:, h, :],
                                    start=(g2 == 0 and h == 0
                                           and oc == 0),
                                    stop=(g2 == 1 and h == H - 1
                                          and oc == NOWN - 1))
                if layer == 2:
                    for o in range(2):
                        nc.vector.tensor_scalar_add(
                            logit_sb[:, :, o], logit_ps[:, :, o],
                            fcb_sb[:, o:o + 1])
                    nc.sync.dma_start(
                        out=out_d.rearrange("(q p) o -> p q o", p=128),
                        in_=logit_sb[:, :, :])
                fin_st.close()

            if layer == 1:
                # ---- h2 rows (+es) per own chunk; scatter exports and
                # AllGather the compact [E, ROWW] block.  Chunk-complete
                # order (s2 inner): chunks 0-2 only need x1T group A, so
                # their h2/export work overlaps the group-B finalize.
                with tc.tile_pool(name="h2", bufs=1, space="PSUM") as h2p:
                    h2_tiles = [h2p.tile([128, HCE], F32, tag=f"h2_{oc}",
                                         name=f"h2_{oc}")
                                for oc in range(IC // 128)]
                    for oc in range(IC // 128):
                        for s2 in range(2):
                            nc.tensor.matmul(
                                h2_tiles[oc][:, :],
                                x1T[:, s2, oc * 128:(oc + 1) * 128],
                                w2p_sb[:, s2, :],
                                start=(s2 == 0), stop=(s2 == 1))
                        if oc % 2 == 0:
                            nc.scalar.copy(
                                g65[:, oc, :, 0:C],
                                h2_tiles[oc][:, 0:HC].rearrange(
                                    "p (h c) -> p h c", h=H))
                        else:
                            nc.vector.tensor_scalar_add(
                                g65[:, oc, :, 0:C],
                                h2_tiles[oc][:, 0:HC].rearrange(
                                    "p (h c) -> p h c", h=H), 0.0)
                        nc.vector.tensor_scalar_add(
                            hg_sb[:, oc, H * (C + 1):ROWW],
                            h2_tiles[oc][:, HC:HCE], 0.0)
                        # exported rows of this chunk -> hgx_dram slots;
                        # non-export lanes carry BIGIDX and are skipped
                        nc.gpsimd.indirect_dma_start(
                            out=hgx_dram[:, :],
                            out_offset=bass.IndirectOffsetOnAxis(
                                ap=scatidx_sb[:, oc:oc + 1], axis=0),
                            in_=hg_sb[:, oc, :], in_offset=None,
                            bounds_check=E - 1, oob_is_err=False)
                        nc.sync.dma_start(
                            out=hg_dram.rearrange(
                                "(q p) r -> p q r", p=128)[:, oc, :],
                            in_=hg_sb[:, oc, :])
                if fake_ag:
                    for r in range(n_cores):
                        nc.sync.dma_start(
                            out=ag_out[r * E:(r + 1) * E, :],
                            in_=hgx_dram[:, :])
                else:
                    nc.gpsimd.collective_compute(
                        "AllGather", OP.bypass,
                        replica_groups=[list(range(n_cores))],
                        ins=[hgx_dram.opt()],
                        outs=[ag_out.opt()])


    nc.compile()
    return nc


_BUILD_CACHE = {}


def _get_nc(nslot, nexp):
    key = (nslot, nexp)
    if key not in _BUILD_CACHE:
        _BUILD_CACHE[key] = build(nslot, nexp)
    return _BUILD_CACHE[key]


def _morton(p, bits=10):
    q = np.clip((p * (1 << bits)).astype(np.int64), 0, (1 << bits) - 1)
    code = np.zeros(len(p), np.int64)
    for b in range(bits):
        for dim in range(3):
            code |= ((q[:, dim] >> b) & 1) << (3 * b + dim)
    return code


def _plan(pts):
    """Sort nodes spatially; build per-core slot tiles.

    Own slots 0-5 are the core's identity Morton blocks restricted to
    their dst windows; "extra" tiles hold remote sources plus own fixup
    nodes (any out-of-window edge), processed over the full dst range
    with an index-only window mask to avoid double counting.
    """
    order = np.argsort(_morton(pts), kind="stable")
    p_sorted = np.full((KP, 3), PAD_COORD, np.float32)
    p_sorted[:K] = pts[order]

    sq = (p_sorted ** 2).sum(-1, dtype=np.float32)
    G = p_sorted @ p_sorted.T
    d2 = sq[None, :] + sq[:, None] - 2.0 * G
    near = d2 < (R2 + MASK_EPS)          # [src, dst], conservative superset

    win = _windows(NOWN)
    extras_list = []
    for c in range(N_CORES):
        base = c * IC
        ncols = near[:, base:base + IC]
        srcs = np.flatnonzero(ncols.any(axis=1))
        rem = srcs[(srcs < base) | (srcs >= base + IC)]
        fix = []
        for s in range(NOWN):
            lo, hi = win[s]
            outside = np.ones(IC, bool)
            outside[lo:hi] = False
            blk = np.arange(base + s * 128, base + (s + 1) * 128)
            viol = ncols[blk] & outside[None, :]
            fix.append(blk[viol.any(axis=1)])
        extras = np.unique(np.concatenate([rem] + fix))
        extras_list.append(extras)
    XT = max(-(-len(e) // 128) for e in extras_list)
    extras_list = [np.concatenate(
        [e, np.full(XT * 128 - len(e), PAD_NODE, np.int64)])
        for e in extras_list]
    T = NOWN + XT
    # export sets: rows of owner o consumed by any OTHER core
    exp_sets = [set() for _ in range(N_CORES)]
    for c in range(N_CORES):
        e = extras_list[c]
        rrem = e[(e != PAD_NODE) & ((e < c * IC) | (e >= (c + 1) * IC))]
        for r in rrem:
            exp_sets[int(r) // IC].add(int(r))
    exp_rows = [np.array(sorted(x), np.int64) for x in exp_sets]
    E = max(8, max(len(x) for x in exp_rows))
    return order, p_sorted, extras_list, T, exp_rows, E


def _blockdiag(a):  # [H, C] -> [HC, H] fp32
    m = np.zeros((HC, H), dtype=np.float32)
    for h in range(H):
        m[h * C:(h + 1) * C, h] = np.asarray(a, np.float32)[h]
    return m


def _prep_inputs(pos, pos_non_manifold, W1, a_src1, a_dst1, b1,
                 W2, a_src2, a_dst2, b2, fc_w, fc_b):
    f16 = np.float16
    pts = np.concatenate([np.asarray(pos, np.float32),
                          np.asarray(pos_non_manifold, np.float32)],
                         axis=2)[0].T  # [K, 3]
    order, p_sorted, extras_list, T, exp_rows, E = _plan(pts)
    XT = T - NOWN
    ET = -(-E // 128)
    win = _windows(NOWN)
    sq_sorted = (p_sorted ** 2).sum(-1, dtype=np.float32)
    # global node id -> AllGather row position (owner-block concat)
    ag_pos = np.full(KP, 0, np.int64)
    for o in range(N_CORES):
        ag_pos[exp_rows[o]] = o * E + np.arange(len(exp_rows[o]))

    W1f = np.asarray(W1, np.float32)
    W2f = np.asarray(W2, np.float32)
    w1s = W1f @ _blockdiag(a_src1)            # [3, H]
    w2p = np.concatenate([W2f, W2f @ _blockdiag(a_src2)], axis=1)

    shared = {
        "w1p": np.ascontiguousarray(W1f),
        "w1d": np.ascontiguousarray(W1f @ _blockdiag(a_dst1)),
        "w2p": np.ascontiguousarray(w2p.astype(f16)),
        "admw2": np.ascontiguousarray(
            (W2f @ _blockdiag(a_dst2)).astype(f16)),
        "b1t": np.ascontiguousarray(
            np.asarray(b1, np.float32).reshape(H, C).T),
        "b2t": np.ascontiguousarray(
            np.asarray(b2, np.float32).reshape(H, C).T),
        "fcw": np.ascontiguousarray(np.asarray(fc_w, np.float32).reshape(
            H, C, 2).transpose(1, 0, 2).astype(f16)),
        "fcb": np.ascontiguousarray(np.broadcast_to(
            np.asarray(fc_b, np.float32).reshape(1, 2), (128, 2))),
    }
    in_maps = []
    for c in range(N_CORES):
        base = c * IC
        own = np.arange(base, base + IC, dtype=np.int64)
        extras = extras_list[c]
        srcs = np.concatenate([own, extras])          # [T*128]
        psel = p_sorted[srcs]                         # [T*128, 3]
        pown = p_sorted[base:base + IC]
        sel5 = np.concatenate(
            [psel.T, sq_sorted[srcs][None, :],
             np.ones((1, len(srcs)), np.float32)], axis=0)
        own5 = np.concatenate(
            [2.0 * pown.T, -np.ones((1, IC), np.float32),
             (R2 - sq_sorted[base:base + IC])[None, :]], axis=0)
        es_cols = np.concatenate(
            [w1s, np.zeros((2, H), np.float32)], axis=0)  # [5, H]
        # index-only window mask for extra-slot lanes: for own fixup
        # lanes, kill the dsts the own slot already covered.
        wmn = np.zeros((XT, 128, IC), np.float16)
        locidx = np.full((XT, 128), BIGIDX, np.int64)
        for t in range(XT):
            for p in range(128):
                n = int(extras[t * 128 + p])
                if n == PAD_NODE:
                    locidx[t, p] = 0          # safe local row, fully masked
                elif base <= n < base + IC:
                    locidx[t, p] = n - base   # own fixup lane
                    lo, hi = win[(n - base) // 128]
                    wmn[t, p, lo:hi] = MNEG
        agidx = ag_pos[srcs].copy()
        isrem = (srcs != PAD_NODE) & ((srcs < base) | (srcs >= base + IC))
        agidx[~isrem] = BIGIDX                # skip non-remote lanes
        m = dict(shared)
        m["sel5"] = np.ascontiguousarray(sel5)
        m["own5ge"] = np.ascontiguousarray(
            np.concatenate([own5, es_cols], axis=1))
        m["own3"] = np.ascontiguousarray(pown.T)
        m["agidx"] = np.ascontiguousarray(
            agidx.reshape(T, 128).T.astype(np.int32))
        m["locidx"] = np.ascontiguousarray(locidx.T.astype(np.int32))
        m["wmn"] = np.ascontiguousarray(wmn.transpose(1, 0, 2))
        # per own chunk: each lane's slot in this core's export block
        # (BIGIDX = not exported, scatter skips it)
        exp_loc = np.full(IC, BIGIDX, np.int64)
        exp_loc[exp_rows[c] - base] = np.arange(len(exp_rows[c]))
        m["scatidx"] = np.ascontiguousarray(
            exp_loc.reshape(NOWN, 128).T.astype(np.int32))
        in_maps.append(m)
    return in_maps, order, T, E


def kernel(pos, pos_non_manifold, W1, a_src1, a_dst1, b1,
           W2, a_src2, a_dst2, b2, fc_w, fc_b, _trace=False):
    in_maps, order, T, E = _prep_inputs(
        pos, pos_non_manifold, W1, a_src1, a_dst1, b1,
        W2, a_src2, a_dst2, b2, fc_w, fc_b)
    nc = _get_nc(T, E)
    res = run_bass_kernel_spmd(nc, in_maps, core_ids=list(range(N_CORES)),
                               trace=_trace)
    kernel.last_results = res
    x2s = np.concatenate([res.results[c]["out"] for c in range(N_CORES)],
                         axis=0)  # [KP, 2] in sorted order
    x2 = np.empty((K, 2), np.float32)
    x2[order] = x2s[:K]
    logits = np.ascontiguousarray(x2[M:K]).reshape(1, 2, 3000)
    return logits.astype(np.float32)


# revision 48
# speedup vs baseline: 1.0612x; 1.0612x over previous
"""Trainium2 Bass kernel for a 2-layer GAT occupancy predictor (B=1).

Reference math:
  pts = concat(pos, pos_non_manifold) -> [K=6000, 3]
  mask[i,j] = ||pts_i - pts_j||^2 < 0.05^2          (dense radius graph)
  layer l:  h = x @ Wl                              [K, 4*64]
            e[i,j,h] = leaky02(ed[i,h] + es[j,h])   es/ed = <h, a_src/dst>
            alpha = softmax_j(e masked)
            x' = relu(alpha @ h + b)
  logits = (x2 @ fc_w + fc_b)[M:] reshaped to [1, 2, 3000]

Distribution (8 NeuronCores): nodes are Morton-sorted; core c owns the 768
destinations [768c, 768(c+1)) of the padded 6144-node graph.

Slot structure per core (T = 6 + XT slots of 128 sources each):
  slots 0-5 : own Morton blocks in identity order.  Each only processes
              the dst WINDOW [128s-128, 128s+256) -- Morton locality puts
              nearly all of a block's edges there (~58% of the dense
              volume).  Out-of-window edges are re-covered by the extra
              slots below.
  slots 6+  : "extra" tiles = remote sources (other cores) plus own
              "fixup" nodes that have any out-of-window edge.  These
              process the full 768-dst range; a host-built, index-only
              wmn mask (-60000 on the in-window range of each fixup
              lane) removes the pairs already covered by the own slots.

Everything 16-bit on the hot path (fp16), f32 accumulation in PSUM.
Aggregation PSUM is k-major [128, 6, H, 128] so each 128-dst chunk k is
one 2KB PSUM bank; the first slot touching chunk k issues start=True on
its head-0 matmul (clearing the whole bank), later heads/slots ride
has_written=0 overwrite/accumulate semantics.

Between layers: x1^T assembled by partition-moving DMAs; h2 = x1 @ W2
(+es ride-along) computed per-owner; only the E exported boundary rows
(the rows some peer actually consumes) are AllGathered as fp16 rows
[h0|1|h1|1|h2|1|h3|1|es4].  Own slots read h2 straight from the
resident hg_sb buffer and overlap the AllGather; extra slots assemble
their source tiles with two bounds-checked indirect gathers (own fixup
lanes from local hg_dram, remote lanes from the AllGather output).
Masks bounce through DRAM between layers.
"""

import sys

sys.path.insert(0, "/opt/trn_rl_repo")

from contextlib import ExitStack

import ml_dtypes
import numpy as np

import concourse.bacc as bacc
import concourse.bass as bass
import concourse.mybir as mybir
import concourse.tile as tile
from concourse.bass_utils import run_bass_kernel_spmd

F32 = mybir.dt.float32
F16 = mybir.dt.float16
I32 = mybir.dt.int32
AF = mybir.ActivationFunctionType
OP = mybir.AluOpType
AX = mybir.AxisListType

N_CORES = 8
N = 3000
M = 3000
K = N + M          # real nodes
KP = 6144          # padded nodes
IC = KP // N_CORES # 768 destinations per core
NOWN = IC // 128   # 6 own slots
H = 4              # heads
C = 64             # channels per head
HC = H * C         # 256
HCE = HC + H       # 260: h columns + es columns (layer-2 ride-along)
ROWW = H * (C + 1) + H  # 264: AG row [h0|1|h1|1|h2|1|h3|1|es4]
R2 = float(np.float32(0.05) * np.float32(0.05))
PAD_COORD = -1.0
PAD_NODE = KP - 1
MASK_EPS = 1e-5    # host activity-test margin (superset of device mask)
MNEG = -60000.0    # masked-score offset; *0.2 then exp -> 0 in fp16
GA = 384           # d2/mask column chunk (PSUM bank budget)
W_LO, W_HI = 128, 256   # own-slot dst window [128s-W_LO, 128s+W_HI)
BIGIDX = 1 << 20   # skip sentinel for bounds-checked indirect gathers


SEG = IC // 2      # extra tiles cover one 384-dst segment each


def _windows(T):
    win = [(max(0, 128 * s - W_LO), min(IC, 128 * s + W_HI))
           for s in range(NOWN)]
    # extra tiles alternate segments [0,384) / [384,768); _plan packs
    # sources to match (tile i covers segment i % 2)
    win += [((i % 2) * SEG, (i % 2) * SEG + SEG) for i in range(T - NOWN)]
    return win


def build(nslot, nexp, n_cores=N_CORES, fake_ag=False):
    nc = bacc.Bacc("TRN2", target_bir_lowering=False, debug=False,
                   num_devices=n_cores)
    T = nslot
    E = nexp
    XT = T - NOWN
    WIN = _windows(T)
    # L1 runs the full-range extra tiles FIRST so the windowed own slots
    # finish each dst chunk k as early as possible -- the finalize for
    # k-group A (chunks 0-2) then overlaps the last own slots.  L2 keeps
    # own slots first (they hide the AllGather) and extras last.
    ORD = {1: list(range(NOWN, T)) + list(range(NOWN)),
           2: list(range(T))}
    first_pos, last_pos = {}, {}
    for layer in (1, 2):
        fp, lp = {}, {}
        for pos, s in enumerate(ORD[layer]):
            lo, hi = WIN[s]
            for k in range(lo // 128, hi // 128):
                if k not in fp:
                    fp[k] = pos
                lp[k] = pos
        first_pos[layer], last_pos[layer] = fp, lp

    # ---- kernel I/O (identical program on every core) ----
    sel5_d = nc.dram_tensor("sel5", [5, T * 128], F32, kind="ExternalInput")
    # own5ge: cols 0:768 = [2p; -1; R2-sq] (g = R2-d2), cols 768:772 = es1
    own5ge_d = nc.dram_tensor("own5ge", [5, IC + H], F32,
                              kind="ExternalInput")
    own3_d = nc.dram_tensor("own3", [3, IC], F32, kind="ExternalInput")
    agidx_d = nc.dram_tensor("agidx", [128, T], I32, kind="ExternalInput")
    locidx_d = nc.dram_tensor("locidx", [128, XT], I32,
                              kind="ExternalInput")
    ET = -(-E // 128)
    expidx_d = nc.dram_tensor("expidx", [128, ET], I32,
                              kind="ExternalInput")
    wmn_d = nc.dram_tensor("wmn", [128, XT, IC], F16, kind="ExternalInput")
    w1p_d = nc.dram_tensor("w1p", [3, HC], F32, kind="ExternalInput")
    w1d_d = nc.dram_tensor("w1d", [3, H], F32, kind="ExternalInput")
    w2p_d = nc.dram_tensor("w2p", [HC, HCE], F16, kind="ExternalInput")
    admw2_d = nc.dram_tensor("admw2", [HC, H], F16, kind="ExternalInput")
    b1t_d = nc.dram_tensor("b1t", [C, H], F32, kind="ExternalInput")
    b2t_d = nc.dram_tensor("b2t", [C, H], F32, kind="ExternalInput")
    fcw_d = nc.dram_tensor("fcw", [C, H, 2], F16, kind="ExternalInput")
    fcb_d = nc.dram_tensor("fcb", [128, 2], F32, kind="ExternalInput")

    out_d = nc.dram_tensor("out", [IC, 2], F32, kind="ExternalOutput")

    # packed per-slot mask column offsets (masks stay resident in SBUF
    # across both layers)
    woff = [0]
    for s in range(T):
        woff.append(woff[-1] + (WIN[s][1] - WIN[s][0]))
    WTOT = woff[-1]

    with tile.TileContext(nc) as tc, ExitStack() as st:
        dram = st.enter_context(tc.tile_pool(name="dram", bufs=1,
                                             space="DRAM"))
        hg_dram = dram.tile([IC, ROWW], F16)
        hgx_dram = dram.tile([E, ROWW], F16)
        ag_out = dram.tile([n_cores * E, ROWW], F16,
                           addr_space=("Local" if fake_ag else "Shared"))

        const = st.enter_context(tc.tile_pool(name="const", bufs=1))
        sel5_sb = const.tile([5, T * 128], F32)
        own5ge_sb = const.tile([5, IC + H], F32)
        own3_sb = const.tile([3, IC], F32)
        agidx_sb = const.tile([128, T], I32)
        locidx_sb = const.tile([128, XT], I32)
        expidx_sb = const.tile([128, ET], I32)
        wmn_sb = const.tile([128, XT, IC], F16)
        w1p_sb = const.tile([3, HC], F32)
        w1d_sb = const.tile([3, H], F32)
        w2p_sb = const.tile([128, 2, HCE], F16)
        admw2_sb = const.tile([128, 2, H], F16)
        b1t_sb = const.tile([C, H], F32)
        b2t_sb = const.tile([C, H], F32)
        fcw_sb = const.tile([C, H, 2], F16)
        fcb_sb = const.tile([128, 2], F32)

        nc.sync.dma_start(out=own3_sb[:, :], in_=own3_d[:, :])
        nc.sync.dma_start(out=w1d_sb[:, :], in_=w1d_d[:, :])
        nc.sync.dma_start(out=sel5_sb[:, :], in_=sel5_d[:, :])
        nc.sync.dma_start(out=own5ge_sb[:, :], in_=own5ge_d[:, :])
        nc.sync.dma_start(out=w1p_sb[:, :], in_=w1p_d[:, :])
        nc.sync.dma_start(out=agidx_sb[:, :], in_=agidx_d[:, :])
        nc.sync.dma_start(out=locidx_sb[:, :], in_=locidx_d[:, :])
        nc.sync.dma_start(out=expidx_sb[:, :], in_=expidx_d[:, :])
        nc.sync.dma_start(out=wmn_sb[:, :, :], in_=wmn_d[:, :, :])
        nc.sync.dma_start(out=w2p_sb[:, :, :],
                          in_=w2p_d.rearrange("(s p) c -> p s c", p=128))
        nc.sync.dma_start(out=admw2_sb[:, :, :],
                          in_=admw2_d.rearrange("(s p) h -> p s h", p=128))
        nc.sync.dma_start(out=b1t_sb[:, :], in_=b1t_d[:, :])
        nc.sync.dma_start(out=b2t_sb[:, :], in_=b2t_d[:, :])
        nc.sync.dma_start(out=fcw_sb[:, :, :], in_=fcw_d[:, :, :])
        nc.sync.dma_start(out=fcb_sb[:, :], in_=fcb_d[:, :])

        big = st.enter_context(tc.tile_pool(name="big", bufs=1))
        # layer-1 source features, AG-row layout [h0|1|h1|1|h2|1|h3|1|es4]
        hsrc = big.tile([128, T, ROWW], F16)
        es4f = big.tile([128, T, H], F32)
        ed_b = big.tile([128, H, IC], F16)
        edt_sb = big.tile([H, IC], F16)
        edt_row = big.tile([1, H, IC], F16)
        x1T = big.tile([128, 2, IC], F16)
        hg_sb = big.tile([128, IC // 128, ROWW], F16)
        xr = big.tile([C, H, IC], F16)
        # k-major finalize staging: flat order (k, h, d)
        den_sb = big.tile([128, NOWN, H, 128], F32)
        dinv_sb = big.tile([128, NOWN, H, 128], F32)
        dinv_row = big.tile([1, NOWN, H, 128], F32)
        dinv_b = big.tile([128, NOWN, H, 128], F32)
        xc = big.tile([C, NOWN, H, 128], F16)
        exp_sb = big.tile([128, ET, ROWW], F16)
        mns = big.tile([128, WTOT], F16)
        logit_sb = big.tile([128, IC // 128, 2], F32)

        h65 = hsrc[:, :, 0:H * (C + 1)].rearrange("p t (h x) -> p t h x", h=H)
        nc.vector.memset(h65[:, :, :, C:C + 1], 1.0)
        g65 = hg_sb[:, :, 0:H * (C + 1)].rearrange("p q (h x) -> p q h x",
                                                   h=H)
        nc.vector.memset(g65[:, :, :, C:C + 1], 1.0)

        for layer in (1, 2):
            # ---- prep: edt rows + partition-broadcast to ed_b ----
            with tc.tile_pool(name=f"prep{layer}", bufs=1,
                              space="PSUM") as prep_ps:
                edt_ps = prep_ps.tile([H, IC], F32, tag="edt")
                # 384-col groups align with the finalize k-groups, so
                # layer-2 edt group A only waits on x1T group A; each
                # group's ed_b broadcast runs as soon as its edt lands.
                for gl in (0, SEG):
                    sl = slice(gl, gl + SEG)
                    if layer == 1:
                        nc.tensor.matmul(edt_ps[:, sl], w1d_sb[:, :],
                                         own3_sb[:, sl],
                                         start=True, stop=True)
                    else:
                        for s2 in range(2):
                            nc.tensor.matmul(edt_ps[:, sl],
                                             admw2_sb[:, s2, :],
                                             x1T[:, s2, sl],
                                             start=(s2 == 0), stop=(s2 == 1))
                    nc.vector.tensor_scalar_add(edt_sb[:, sl],
                                                edt_ps[:, sl], 0.0)
                    nc.sync.dma_start(out=edt_row[0:1, :, sl],
                                      in_=edt_sb[:, sl])
                    nc.gpsimd.partition_broadcast(
                        ed_b[:, :, sl], edt_row[0:1, :, sl])

            # ---- slot loop ----
            with tc.tile_pool(name=f"agg_ps{layer}", bufs=1,
                              space="PSUM") as agg_pool:
                # k-major: chunk k of 128 dsts = one 2KB PSUM bank
                agg_ps = agg_pool.tile([128, NOWN, H, 128], F32, tag="agg",
                                       name=f"agg_{layer}")
                with tc.tile_pool(name=f"jl{layer}", bufs=4) as jl, \
                     tc.tile_pool(name=f"h_ps{layer}", bufs=1,
                                  space="PSUM") as h_psp:
                    for pos, s in enumerate(ORD[layer]):
                        lo, hi = WIN[s]
                        Ws = hi - lo
                        mn = mns[:, woff[s]:woff[s] + Ws]
                        if layer == 1:
                            h_ps = h_psp.tile([128, HC], F32, tag="h",
                                              name=f"h_ps_{s}")
                            nc.tensor.matmul(
                                h_ps[:, :],
                                sel5_sb[0:3, s * 128:(s + 1) * 128],
                                w1p_sb[:, :], start=True, stop=True)
                            nc.scalar.copy(
                                h65[:, s, :, 0:C],
                                h_ps[:, :].rearrange("p (h c) -> p h c",
                                                     h=H))
                            g_ps = h_psp.tile([128, GA + H], F32, tag="g",
                                              name=f"g_ps_{s}")
                            # one windowed g chunk + es ride columns
                            nc.tensor.matmul(
                                g_ps[:, 0:Ws],
                                sel5_sb[:, s * 128:(s + 1) * 128],
                                own5ge_sb[:, lo:hi],
                                start=True, stop=True)
                            nc.tensor.matmul(
                                g_ps[:, GA:GA + H],
                                sel5_sb[:, s * 128:(s + 1) * 128],
                                own5ge_sb[:, IC:IC + H],
                                start=True, stop=True)
                            nc.vector.tensor_scalar(
                                mn[:, 0:Ws], g_ps[:, 0:Ws], 0.0, MNEG,
                                OP.is_lt, OP.mult)
                            nc.vector.tensor_scalar_add(
                                es4f[:, s, :], g_ps[:, GA:GA + H], 0.0)
                            if s >= NOWN:
                                # index-only mask: remove the pairs the
                                # own slots already covered
                                nc.vector.tensor_tensor(
                                    mn[:, 0:Ws], mn[:, 0:Ws],
                                    wmn_sb[:, s - NOWN, lo:hi], OP.min)
                            src = hsrc[:, s, :]
                            es_ap = es4f[:, s, :]
                        else:
                            if s < NOWN:
                                # own-node slots: h2 rows resident in hg_sb
                                src = hg_sb[:, s, :]
                            else:
                                src = jl.tile([128, ROWW], F16, tag="hg",
                                              name=f"hg_{s}")
                                # fixup lanes from local hg rows (early,
                                # no AllGather dependency)...
                                nc.gpsimd.indirect_dma_start(
                                    out=src[:, :], out_offset=None,
                                    in_=hg_dram[:, :],
                                    in_offset=bass.IndirectOffsetOnAxis(
                                        ap=locidx_sb[:, s - NOWN:s - NOWN + 1],
                                        axis=0),
                                    bounds_check=IC - 1, oob_is_err=False)
                                # ...remote lanes from the AllGather
                                nc.gpsimd.indirect_dma_start(
                                    out=src[:, :], out_offset=None,
                                    in_=ag_out[:, :],
                                    in_offset=bass.IndirectOffsetOnAxis(
                                        ap=agidx_sb[:, s:s + 1], axis=0),
                                    bounds_check=n_cores * E - 1,
                                    oob_is_err=False)
                            esg = jl.tile([128, H], F32, tag="esg",
                                          name=f"esg_{s}")
                            nc.vector.tensor_scalar_add(
                                esg[:, :],
                                src[:, H * (C + 1):ROWW], 0.0)
                            es_ap = esg[:, :]

                        # scores: L = leaky02(ed + es + mn); A = exp(L).
                        # u4 = ed + mn in ONE 2x TT via a stride-0 head
                        # broadcast of mn.  Then heads 0-1 get es+leaky via
                        # ACT Prelu (bias=es); heads 2-3 via 4x TS es-adds
                        # and a TS/TT leaky (STT only has a 1x uop).
                        L4 = jl.tile([128, H, IC], F16, tag="L4",
                                     name=f"L4_{layer}_{s}")
                        u4 = jl.tile([128, H, IC], F16, tag="u4",
                                     name=f"u4_{layer}_{s}")
                        ub, mb = bass.broadcast_tensor_aps(
                            ed_b[:, :, lo:hi],
                            mn[:, 0:Ws].rearrange("p (o d) -> p o d", o=1))
                        nc.vector.tensor_tensor(u4[:, :, 0:Ws], ub, mb,
                                                OP.add)
                        for h in range(2):
                            nc.scalar.activation(
                                L4[:, h, 0:Ws], u4[:, h, 0:Ws], AF.Prelu,
                                bias=es_ap[:, h:h + 1],
                                scale=1.0, alpha=0.2)
                        v2 = jl.tile([128, 2, IC], F16, tag="v2",
                                     name=f"v2_{layer}_{s}")
                        for h in range(2, H):
                            nc.vector.tensor_scalar_add(
                                v2[:, h - 2, 0:Ws], u4[:, h, 0:Ws],
                                es_ap[:, h:h + 1])
                        t2 = jl.tile([128, 2, IC], F16, tag="t2",
                                     name=f"t2_{layer}_{s}")
                        nc.vector.tensor_scalar_mul(t2[:, :, 0:Ws],
                                                    v2[:, :, 0:Ws], 0.2)
                        nc.vector.tensor_tensor(L4[:, 2:4, 0:Ws],
                                                v2[:, :, 0:Ws],
                                                t2[:, :, 0:Ws], OP.max)
                        A4 = jl.tile([128, H, IC], F16, tag="A4",
                                     name=f"A4_{layer}_{s}")
                        nc.scalar.activation(A4[:, :, 0:Ws],
                                             L4[:, :, 0:Ws], AF.Exp)

                        # transposed aggregation: [h|ones] stationary,
                        # one 128-dst chunk (=one PSUM bank) per matmul.
                        # start=True (bank clear) only on the head-0
                        # matmul of the first slot touching chunk k.
                        for h in range(H):
                            for k in range(lo // 128, hi // 128):
                                nc.tensor.matmul(
                                    agg_ps[0:C + 1, k, h, :],
                                    src[:, h * (C + 1):(h + 1) * (C + 1)],
                                    A4[:, h, k * 128 - lo:k * 128 - lo + 128],
                                    start=(pos == first_pos[layer][k]
                                           and h == 0),
                                    stop=(pos == last_pos[layer][k]))

                # ---- finalize: x^T = relu(num*dinv + b) ----
                bt_sb = b1t_sb if layer == 1 else b2t_sb
                xT = x1T
                # k-group split: group A (chunks 0-2) stops accumulating
                # before group B in layer 1 (extras-first order), so its
                # whole den->dinv->broadcast->mult chain overlaps the
                # remaining own slots.
                KH = [slice(0, NOWN // 2), slice(NOWN // 2, NOWN)]
                fin_st = ExitStack()
                if layer == 2:
                    fc_ps_pool = fin_st.enter_context(
                        tc.tile_pool(name="fc", bufs=1, space="PSUM"))
                    logit_ps = fc_ps_pool.tile([128, IC // 128, 2], F32,
                                               tag="lg")
                # 1/den as exp(-ln(den)) straight from PSUM.  Strict
                # phase order Ln,Ln,Exp,Exp costs exactly two table
                # loads per layer (interleaving the groups would thrash
                # the ACT table back and forth).
                for g2 in range(2):
                    nc.scalar.activation(
                        den_sb[C:C + 1, KH[g2], :, :],
                        agg_ps[C:C + 1, KH[g2], :, :],
                        AF.Ln)
                for g2 in range(2):
                    nc.scalar.activation(
                        dinv_sb[C:C + 1, KH[g2], :, :],
                        den_sb[C:C + 1, KH[g2], :, :],
                        AF.Exp, scale=-1.0)
                for g2 in range(2):
                    ks = KH[g2]
                    nc.sync.dma_start(
                        out=dinv_row[0:1, ks, :, :],
                        in_=dinv_sb[C:C + 1, ks, :, :])
                    nc.gpsimd.partition_broadcast(
                        dinv_b[0:C, ks, :, :].rearrange(
                            "p k h d -> p (k h d)"),
                        dinv_row[0:1, ks, :, :].rearrange(
                            "p k h d -> p (k h d)"))
                    nc.vector.tensor_tensor(
                        xc[:, ks, :, :], agg_ps[0:C, ks, :, :],
                        dinv_b[0:C, ks, :, :], OP.mult)
                    gcol = slice(g2 * (IC // 2), (g2 + 1) * (IC // 2))
                    for h in range(H):
                        nc.vector.tensor_scalar(
                            xr[:, h, gcol].rearrange("p (k d) -> p k d",
                                                     k=NOWN // 2),
                            xc[:, ks, h, :], bt_sb[:, h:h + 1],
                            0.0, OP.add, OP.max)
                        if layer == 1:
                            po = (h % 2) * C
                            nc.sync.dma_start(
                                out=xT[po:po + C, h // 2, gcol],
                                in_=xr[0:C, h, gcol])
                        if layer == 2:
                            for oc in range(g2 * (NOWN // 2),
                                            (g2 + 1) * (NOWN // 2)):
                                nc.tensor.matmul(
                                    logit_ps[:, oc, :],
                                    xr[0:C, h, oc * 128:(oc + 1) * 128],
                                    fcw_sb[:, h, :],
                                    start=(g2 == 0 and h == 0
                                           and oc == 0),
                                    stop=(g2 == 1 and h == H - 1
                                          and oc == NOWN - 1))
                if layer == 2:
                    for o in range(2):
                        nc.vector.tensor_scalar_add(
                            logit_sb[:, :, o], logit_ps[:, :, o],
                            fcb_sb[:, o:o + 1])
                    nc.sync.dma_start(
                        out=out_d.rearrange("(q p) o -> p q o", p=128),
                        in_=logit_sb[:, :, :])
                fin_st.close()

            if layer == 1:
                # ---- h2 rows (+es) per own chunk; scatter exports and
                # AllGather the compact [E, ROWW] block.  Chunk-complete
                # order (s2 inner): chunks 0-2 only need x1T group A, so
                # their h2/export work overlaps the group-B finalize.
                with tc.tile_pool(name="h2", bufs=1, space="PSUM") as h2p:
                    h2_tiles = [h2p.tile([128, HCE], F32, tag=f"h2_{oc}",
                                         name=f"h2_{oc}")
                                for oc in range(IC // 128)]
                    for oc in range(IC // 128):
                        for s2 in range(2):
                            nc.tensor.matmul(
                                h2_tiles[oc][:, :],
                                x1T[:, s2, oc * 128:(oc + 1) * 128],
                                w2p_sb[:, s2, :],
                                start=(s2 == 0), stop=(s2 == 1))
                        if oc % 2 == 0:
                            nc.scalar.copy(
                                g65[:, oc, :, 0:C],
                                h2_tiles[oc][:, 0:HC].rearrange(
                                    "p (h c) -> p h c", h=H))
                        else:
                            nc.vector.tensor_scalar_add(
                                g65[:, oc, :, 0:C],
                                h2_tiles[oc][:, 0:HC].rearrange(
                                    "p (h c) -> p h c", h=H), 0.0)
                        nc.vector.tensor_scalar_add(
                            hg_sb[:, oc, H * (C + 1):ROWW],
                            h2_tiles[oc][:, HC:HCE], 0.0)
                        nc.sync.dma_start(
                            out=hg_dram.rearrange(
                                "(q p) r -> p q r", p=128)[:, oc, :],
                            in_=hg_sb[:, oc, :])
                # gather the E exported rows into the compact AG input
                # (2 gpsimd ops; the per-oc hg_dram DMAs above run on
                # the parallel DMA queues, keeping gpsimd free)
                for t in range(ET):
                    nc.gpsimd.indirect_dma_start(
                        out=exp_sb[:, t, :], out_offset=None,
                        in_=hg_dram[:, :],
                        in_offset=bass.IndirectOffsetOnAxis(
                            ap=expidx_sb[:, t:t + 1], axis=0))
                for t in range(ET):
                    r0 = t * 128
                    r1 = min(E, r0 + 128)
                    nc.sync.dma_start(out=hgx_dram[r0:r1, :],
                                      in_=exp_sb[0:r1 - r0, t, :])
                if fake_ag:
                    for r in range(n_cores):
                        nc.sync.dma_start(
                            out=ag_out[r * E:(r + 1) * E, :],
                            in_=hgx_dram[:, :])
                else:
                    nc.gpsimd.collective_compute(
                        "AllGather", OP.bypass,
                        replica_groups=[list(range(n_cores))],
                        ins=[hgx_dram.opt()],
                        outs=[ag_out.opt()])


    nc.compile()
    return nc


_BUILD_CACHE = {}


def _get_nc(nslot, nexp):
    key = (nslot, nexp)
    if key not in _BUILD_CACHE:
        _BUILD_CACHE[key] = build(nslot, nexp)
    return _BUILD_CACHE[key]


def _morton(p, bits=10):
    q = np.clip((p * (1 << bits)).astype(np.int64), 0, (1 << bits) - 1)
    code = np.zeros(len(p), np.int64)
    for b in range(bits):
        for dim in range(3):
            code |= ((q[:, dim] >> b) & 1) << (3 * b + dim)
    return code


def _plan(pts):
    """Sort nodes spatially; build per-core slot tiles.

    Own slots 0-5 are the core's identity Morton blocks restricted to
    their dst windows; "extra" tiles hold remote sources plus own fixup
    nodes (any out-of-window edge), processed over the full dst range
    with an index-only window mask to avoid double counting.
    """
    order = np.argsort(_morton(pts), kind="stable")
    p_sorted = np.full((KP, 3), PAD_COORD, np.float32)
    p_sorted[:K] = pts[order]

    sq = (p_sorted ** 2).sum(-1, dtype=np.float32)
    G = p_sorted @ p_sorted.T
    d2 = sq[None, :] + sq[:, None] - 2.0 * G
    near = d2 < (R2 + MASK_EPS)          # [src, dst], conservative superset

    win = _windows(NOWN)
    # per-core, per-384-segment source sets: a source belongs to segment
    # g if it has an "extra-responsibility" edge there (remote edge, or
    # own-fixup edge outside the node's own window)
    seg_sets = []
    for c in range(N_CORES):
        base = c * IC
        ncols = near[:, base:base + IC]
        srcs = np.flatnonzero(ncols.any(axis=1))
        per_seg = [set(), set()]
        for n in srcs:
            if base <= n < base + IC:
                lo, hi = win[(n - base) // 128]
                dsts = np.flatnonzero(ncols[n])
                dsts = dsts[(dsts < lo) | (dsts >= hi)]
            else:
                dsts = np.flatnonzero(ncols[n])
            for g in set(int(dd) // SEG for dd in dsts):
                per_seg[g].add(int(n))
        seg_sets.append([np.array(sorted(x), np.int64) for x in per_seg])
    tiles_per_seg = [max(-(-len(seg_sets[c][g]) // 128)
                         for c in range(N_CORES)) for g in range(2)]
    # extra tiles alternate segments A,B,A,B to match _windows
    assert tiles_per_seg[0] == tiles_per_seg[1], tiles_per_seg
    XT = tiles_per_seg[0] + tiles_per_seg[1]
    extras_list = []
    for c in range(N_CORES):
        tiles = []
        for i in range(XT):
            g = i % 2
            j = i // 2
            e = seg_sets[c][g][j * 128:(j + 1) * 128]
            tiles.append(np.concatenate(
                [e, np.full(128 - len(e), PAD_NODE, np.int64)]))
        extras_list.append(np.concatenate(tiles))
    T = NOWN + XT
    # export sets: rows of owner o consumed by any OTHER core
    exp_sets = [set() for _ in range(N_CORES)]
    for c in range(N_CORES):
        e = extras_list[c]
        rrem = e[(e != PAD_NODE) & ((e < c * IC) | (e >= (c + 1) * IC))]
        for r in rrem:
            exp_sets[int(r) // IC].add(int(r))
    exp_rows = [np.array(sorted(x), np.int64) for x in exp_sets]
    E = max(8, max(len(x) for x in exp_rows))
    return order, p_sorted, extras_list, T, exp_rows, E


def _blockdiag(a):  # [H, C] -> [HC, H] fp32
    m = np.zeros((HC, H), dtype=np.float32)
    for h in range(H):
        m[h * C:(h + 1) * C, h] = np.asarray(a, np.float32)[h]
    return m


def _prep_inputs(pos, pos_non_manifold, W1, a_src1, a_dst1, b1,
                 W2, a_src2, a_dst2, b2, fc_w, fc_b):
    f16 = np.float16
    pts = np.concatenate([np.asarray(pos, np.float32),
                          np.asarray(pos_non_manifold, np.float32)],
                         axis=2)[0].T  # [K, 3]
    order, p_sorted, extras_list, T, exp_rows, E = _plan(pts)
    XT = T - NOWN
    ET = -(-E // 128)
    win = _windows(NOWN)
    sq_sorted = (p_sorted ** 2).sum(-1, dtype=np.float32)
    # global node id -> AllGather row position (owner-block concat)
    ag_pos = np.full(KP, 0, np.int64)
    for o in range(N_CORES):
        ag_pos[exp_rows[o]] = o * E + np.arange(len(exp_rows[o]))

    W1f = np.asarray(W1, np.float32)
    W2f = np.asarray(W2, np.float32)
    w1s = W1f @ _blockdiag(a_src1)            # [3, H]
    w2p = np.concatenate([W2f, W2f @ _blockdiag(a_src2)], axis=1)

    shared = {
        "w1p": np.ascontiguousarray(W1f),
        "w1d": np.ascontiguousarray(W1f @ _blockdiag(a_dst1)),
        "w2p": np.ascontiguousarray(w2p.astype(f16)),
        "admw2": np.ascontiguousarray(
            (W2f @ _blockdiag(a_dst2)).astype(f16)),
        "b1t": np.ascontiguousarray(
            np.asarray(b1, np.float32).reshape(H, C).T),
        "b2t": np.ascontiguousarray(
            np.asarray(b2, np.float32).reshape(H, C).T),
        "fcw": np.ascontiguousarray(np.asarray(fc_w, np.float32).reshape(
            H, C, 2).transpose(1, 0, 2).astype(f16)),
        "fcb": np.ascontiguousarray(np.broadcast_to(
            np.asarray(fc_b, np.float32).reshape(1, 2), (128, 2))),
    }
    in_maps = []
    for c in range(N_CORES):
        base = c * IC
        own = np.arange(base, base + IC, dtype=np.int64)
        extras = extras_list[c]
        srcs = np.concatenate([own, extras])          # [T*128]
        psel = p_sorted[srcs]                         # [T*128, 3]
        pown = p_sorted[base:base + IC]
        sel5 = np.concatenate(
            [psel.T, sq_sorted[srcs][None, :],
             np.ones((1, len(srcs)), np.float32)], axis=0)
        own5 = np.concatenate(
            [2.0 * pown.T, -np.ones((1, IC), np.float32),
             (R2 - sq_sorted[base:base + IC])[None, :]], axis=0)
        es_cols = np.concatenate(
            [w1s, np.zeros((2, H), np.float32)], axis=0)  # [5, H]
        # index-only window mask for extra-slot lanes: for own fixup
        # lanes, kill the dsts the own slot already covered.
        wmn = np.zeros((XT, 128, IC), np.float16)
        locidx = np.full((XT, 128), BIGIDX, np.int64)
        for t in range(XT):
            for p in range(128):
                n = int(extras[t * 128 + p])
                if n == PAD_NODE:
                    locidx[t, p] = 0          # safe local row, fully masked
                elif base <= n < base + IC:
                    locidx[t, p] = n - base   # own fixup lane
                    lo, hi = win[(n - base) // 128]
                    wmn[t, p, lo:hi] = MNEG
        agidx = ag_pos[srcs].copy()
        isrem = (srcs != PAD_NODE) & ((srcs < base) | (srcs >= base + IC))
        agidx[~isrem] = BIGIDX                # skip non-remote lanes
        m = dict(shared)
        m["sel5"] = np.ascontiguousarray(sel5)
        m["own5ge"] = np.ascontiguousarray(
            np.concatenate([own5, es_cols], axis=1))
        m["own3"] = np.ascontiguousarray(pown.T)
        m["agidx"] = np.ascontiguousarray(
            agidx.reshape(T, 128).T.astype(np.int32))
        m["locidx"] = np.ascontiguousarray(locidx.T.astype(np.int32))
        m["wmn"] = np.ascontiguousarray(wmn.transpose(1, 0, 2))
        eloc = np.concatenate(
            [exp_rows[c] - base,
             np.zeros(ET * 128 - len(exp_rows[c]), np.int64)])
        m["expidx"] = np.ascontiguousarray(
            eloc.reshape(ET, 128).T.astype(np.int32))
        in_maps.append(m)
    return in_maps, order, T, E


def kernel(pos, pos_non_manifold, W1, a_src1, a_dst1, b1,
           W2, a_src2, a_dst2, b2, fc_w, fc_b, _trace=False):
    in_maps, order, T, E = _prep_inputs(
        pos, pos_non_manifold, W1, a_src1, a_dst1, b1,
        W2, a_src2, a_dst2, b2, fc_w, fc_b)
    nc = _get_nc(T, E)
    res = run_bass_kernel_spmd(nc, in_maps, core_ids=list(range(N_CORES)),
                               trace=_trace)
    kernel.last_results = res
    x2s = np.concatenate([res.results[c]["out"] for c in range(N_CORES)],
                         axis=0)  # [KP, 2] in sorted order
    x2 = np.empty((K, 2), np.float32)
    x2[order] = x2s[:K]
    logits = np.ascontiguousarray(x2[M:K]).reshape(1, 2, 3000)
    return logits.astype(np.float32)


# revision 50
# speedup vs baseline: 1.3573x; 1.2789x over previous
"""Trainium2 Bass kernel for a 2-layer GAT occupancy predictor (B=1).

Reference math:
  pts = concat(pos, pos_non_manifold) -> [K=6000, 3]
  mask[i,j] = ||pts_i - pts_j||^2 < 0.05^2          (dense radius graph)
  layer l:  h = x @ Wl                              [K, 4*64]
            e[i,j,h] = leaky02(ed[i,h] + es[j,h])   es/ed = <h, a_src/dst>
            alpha = softmax_j(e masked)
            x' = relu(alpha @ h + b)
  logits = (x2 @ fc_w + fc_b)[M:] reshaped to [1, 2, 3000]

Distribution (8 NeuronCores): nodes are Morton-sorted; core c owns the 768
destinations [768c, 768(c+1)) of the padded 6144-node graph.

Slot structure per core (T = 6 + XT slots of 128 sources each):
  slots 0-5 : own Morton blocks in identity order.  Each only processes
              the dst WINDOW [128s-128, 128s+256) -- Morton locality puts
              nearly all of a block's edges there (~58% of the dense
              volume).  Out-of-window edges are re-covered by the extra
              slots below.
  slots 6+  : "extra" tiles = remote sources (other cores) plus own
              "fixup" nodes that have any out-of-window edge.  These
              process the full 768-dst range; a host-built, index-only
              wmn mask (-60000 on the in-window range of each fixup
              lane) removes the pairs already covered by the own slots.

Everything 16-bit on the hot path (fp16), f32 accumulation in PSUM.
Aggregation PSUM is k-major [128, 6, H, 128] so each 128-dst chunk k is
one 2KB PSUM bank; the first slot touching chunk k issues start=True on
its head-0 matmul (clearing the whole bank), later heads/slots ride
has_written=0 overwrite/accumulate semantics.

Between layers: x1^T assembled by partition-moving DMAs; h2 = x1 @ W2
(+es ride-along) computed per-owner; only the E exported boundary rows
(the rows some peer actually consumes) are AllGathered as fp16 rows
[h0|1|h1|1|h2|1|h3|1|es4].  Own slots read h2 straight from the
resident hg_sb buffer and overlap the AllGather; extra slots assemble
their source tiles with two bounds-checked indirect gathers (own fixup
lanes from local hg_dram, remote lanes from the AllGather output).
Masks bounce through DRAM between layers.
"""

import sys

sys.path.insert(0, "/opt/trn_rl_repo")

from contextlib import ExitStack

import ml_dtypes
import numpy as np

import concourse.bacc as bacc
import concourse.bass as bass
import concourse.mybir as mybir
import concourse.tile as tile
from concourse.bass_utils import run_bass_kernel_spmd

F32 = mybir.dt.float32
F16 = mybir.dt.float16
I32 = mybir.dt.int32
AF = mybir.ActivationFunctionType
OP = mybir.AluOpType
AX = mybir.AxisListType

N_CORES = 8
N = 3000
M = 3000
K = N + M          # real nodes
KP = 6144          # padded nodes
IC = KP // N_CORES # 768 destinations per core
NOWN = IC // 128   # 6 own slots
H = 4              # heads
C = 64             # channels per head
HC = H * C         # 256
HCE = HC + H       # 260: h columns + es columns (layer-2 ride-along)
ROWW = H * (C + 1) + H  # 264: AG row [h0|1|h1|1|h2|1|h3|1|es4]
R2 = float(np.float32(0.05) * np.float32(0.05))
PAD_COORD = -1.0
PAD_NODE = KP - 1
MASK_EPS = 1e-5    # host activity-test margin (superset of device mask)
MNEG = -60000.0    # masked-score offset; *0.2 then exp -> 0 in fp16
GA = 384           # d2/mask column chunk (PSUM bank budget)
W_LO, W_HI = 128, 256   # own-slot dst window [128s-W_LO, 128s+W_HI)
BIGIDX = 1 << 20   # skip sentinel for bounds-checked indirect gathers


SEG = IC // 2      # extra tiles cover one 384-dst segment each


def _windows(T):
    win = [(max(0, 128 * s - W_LO), min(IC, 128 * s + W_HI))
           for s in range(NOWN)]
    # extra tiles alternate segments [0,384) / [384,768); _plan packs
    # sources to match (tile i covers segment i % 2)
    win += [((i % 2) * SEG, (i % 2) * SEG + SEG) for i in range(T - NOWN)]
    return win


def build(nslot, nexp, n_cores=N_CORES, fake_ag=False):
    nc = bacc.Bacc("TRN2", target_bir_lowering=False, debug=False,
                   num_devices=n_cores)
    T = nslot
    E = nexp
    XT = T - NOWN
    WIN = _windows(T)
    # L1 runs the full-range extra tiles FIRST so the windowed own slots
    # finish each dst chunk k as early as possible -- the finalize for
    # k-group A (chunks 0-2) then overlaps the last own slots.  L2 keeps
    # own slots first (they hide the AllGather) and extras last.
    ORD = {1: list(range(NOWN, T)) + list(range(NOWN)),
           2: list(range(T))}
    first_pos, last_pos = {}, {}
    for layer in (1, 2):
        fp, lp = {}, {}
        for pos, s in enumerate(ORD[layer]):
            lo, hi = WIN[s]
            for k in range(lo // 128, hi // 128):
                if k not in fp:
                    fp[k] = pos
                lp[k] = pos
        first_pos[layer], last_pos[layer] = fp, lp

    # ---- kernel I/O (identical program on every core) ----
    sel5_d = nc.dram_tensor("sel5", [5, T * 128], F32, kind="ExternalInput")
    # own5ge: cols 0:768 = [2p; -1; R2-sq] (g = R2-d2), cols 768:772 = es1
    own5ge_d = nc.dram_tensor("own5ge", [5, IC + H], F32,
                              kind="ExternalInput")
    own3_d = nc.dram_tensor("own3", [3, IC], F32, kind="ExternalInput")
    agidx_d = nc.dram_tensor("agidx", [128, T], I32, kind="ExternalInput")
    locidx_d = nc.dram_tensor("locidx", [128, XT], I32,
                              kind="ExternalInput")
    ET = -(-E // 128)
    expidx_d = nc.dram_tensor("expidx", [128, ET], I32,
                              kind="ExternalInput")
    wmn_d = nc.dram_tensor("wmn", [128, XT, IC], F16, kind="ExternalInput")
    w1p_d = nc.dram_tensor("w1p", [3, HC], F32, kind="ExternalInput")
    w1d_d = nc.dram_tensor("w1d", [3, H], F32, kind="ExternalInput")
    w2p_d = nc.dram_tensor("w2p", [HC, HCE], F16, kind="ExternalInput")
    admw2_d = nc.dram_tensor("admw2", [HC, H], F16, kind="ExternalInput")
    b1t_d = nc.dram_tensor("b1t", [C, H], F32, kind="ExternalInput")
    b2t_d = nc.dram_tensor("b2t", [C, H], F32, kind="ExternalInput")
    fcw_d = nc.dram_tensor("fcw", [C, H, 2], F16, kind="ExternalInput")
    fcb_d = nc.dram_tensor("fcb", [128, 2], F32, kind="ExternalInput")

    out_d = nc.dram_tensor("out", [IC, 2], F32, kind="ExternalOutput")

    # packed per-slot mask column offsets (masks stay resident in SBUF
    # across both layers)
    woff = [0]
    for s in range(T):
        woff.append(woff[-1] + (WIN[s][1] - WIN[s][0]))
    WTOT = woff[-1]

    with tile.TileContext(nc) as tc, ExitStack() as st:
        dram = st.enter_context(tc.tile_pool(name="dram", bufs=1,
                                             space="DRAM"))
        hg_dram = dram.tile([IC, ROWW], F16)
        hgx_dram = dram.tile([E, ROWW], F16)
        ag_out = dram.tile([n_cores * E, ROWW], F16,
                           addr_space=("Local" if fake_ag else "Shared"))
        warm_in = dram.tile([8, 16], F16)
        warm_out = dram.tile([n_cores * 8, 16], F16,
                             addr_space=("Local" if fake_ag else "Shared"))

        const = st.enter_context(tc.tile_pool(name="const", bufs=1))
        sel5_sb = const.tile([5, T * 128], F32)
        own5ge_sb = const.tile([5, IC + H], F32)
        own3_sb = const.tile([3, IC], F32)
        agidx_sb = const.tile([128, T], I32)
        locidx_sb = const.tile([128, XT], I32)
        expidx_sb = const.tile([128, ET], I32)
        wmn_sb = const.tile([128, XT, IC], F16)
        w1p_sb = const.tile([3, HC], F32)
        w1d_sb = const.tile([3, H], F32)
        w2p_sb = const.tile([128, 2, HCE], F16)
        admw2_sb = const.tile([128, 2, H], F16)
        b1t_sb = const.tile([C, H], F32)
        b2t_sb = const.tile([C, H], F32)
        fcw_sb = const.tile([C, H, 2], F16)
        fcb_sb = const.tile([128, 2], F32)

        # warm up the collective engine/channels at t=0 so the real
        # AllGather's launch doesn't pay one-time mesh setup
        if not fake_ag:
            nc.gpsimd.collective_compute(
                "AllGather", OP.bypass,
                replica_groups=[list(range(n_cores))],
                ins=[warm_in.opt()],
                outs=[warm_out.opt()])

        nc.sync.dma_start(out=own3_sb[:, :], in_=own3_d[:, :])
        nc.sync.dma_start(out=w1d_sb[:, :], in_=w1d_d[:, :])
        nc.sync.dma_start(out=sel5_sb[:, :], in_=sel5_d[:, :])
        nc.sync.dma_start(out=own5ge_sb[:, :], in_=own5ge_d[:, :])
        nc.sync.dma_start(out=w1p_sb[:, :], in_=w1p_d[:, :])
        nc.sync.dma_start(out=agidx_sb[:, :], in_=agidx_d[:, :])
        nc.sync.dma_start(out=locidx_sb[:, :], in_=locidx_d[:, :])
        nc.sync.dma_start(out=expidx_sb[:, :], in_=expidx_d[:, :])
        nc.sync.dma_start(out=wmn_sb[:, :, :], in_=wmn_d[:, :, :])
        nc.sync.dma_start(out=w2p_sb[:, :, :],
                          in_=w2p_d.rearrange("(s p) c -> p s c", p=128))
        nc.sync.dma_start(out=admw2_sb[:, :, :],
                          in_=admw2_d.rearrange("(s p) h -> p s h", p=128))
        nc.sync.dma_start(out=b1t_sb[:, :], in_=b1t_d[:, :])
        nc.sync.dma_start(out=b2t_sb[:, :], in_=b2t_d[:, :])
        nc.sync.dma_start(out=fcw_sb[:, :, :], in_=fcw_d[:, :, :])
        nc.sync.dma_start(out=fcb_sb[:, :], in_=fcb_d[:, :])

        big = st.enter_context(tc.tile_pool(name="big", bufs=1))
        # layer-1 source features, AG-row layout [h0|1|h1|1|h2|1|h3|1|es4]
        hsrc = big.tile([128, T, ROWW], F16)
        es4f = big.tile([128, T, H], F32)
        ed_b = big.tile([128, H, IC], F16)
        edt_sb = big.tile([H, IC], F16)
        edt_row = big.tile([1, H, IC], F16)
        x1T = big.tile([128, 2, IC], F16)
        hg_sb = big.tile([128, IC // 128, ROWW], F16)
        xr = big.tile([C, H, IC], F16)
        # k-major finalize staging: flat order (k, h, d)
        den_sb = big.tile([128, NOWN, H, 128], F32)
        dinv_sb = big.tile([128, NOWN, H, 128], F32)
        dinv_row = big.tile([1, NOWN, H, 128], F32)
        dinv_b = big.tile([128, NOWN, H, 128], F32)
        xc = big.tile([C, NOWN, H, 128], F16)
        exp_sb = big.tile([128, ET, ROWW], F16)
        mns = big.tile([128, WTOT], F16)
        logit_sb = big.tile([128, IC // 128, 2], F32)

        h65 = hsrc[:, :, 0:H * (C + 1)].rearrange("p t (h x) -> p t h x", h=H)
        nc.vector.memset(h65[:, :, :, C:C + 1], 1.0)
        g65 = hg_sb[:, :, 0:H * (C + 1)].rearrange("p q (h x) -> p q h x",
                                                   h=H)
        nc.vector.memset(g65[:, :, :, C:C + 1], 1.0)

        for layer in (1, 2):
            # ---- prep: edt rows + partition-broadcast to ed_b ----
            with tc.tile_pool(name=f"prep{layer}", bufs=1,
                              space="PSUM") as prep_ps:
                edt_ps = prep_ps.tile([H, IC], F32, tag="edt")
                # 384-col groups align with the finalize k-groups, so
                # layer-2 edt group A only waits on x1T group A; each
                # group's ed_b broadcast runs as soon as its edt lands.
                for gl in (0, SEG):
                    sl = slice(gl, gl + SEG)
                    if layer == 1:
                        nc.tensor.matmul(edt_ps[:, sl], w1d_sb[:, :],
                                         own3_sb[:, sl],
                                         start=True, stop=True)
                    else:
                        for s2 in range(2):
                            nc.tensor.matmul(edt_ps[:, sl],
                                             admw2_sb[:, s2, :],
                                             x1T[:, s2, sl],
                                             start=(s2 == 0), stop=(s2 == 1))
                    nc.vector.tensor_scalar_add(edt_sb[:, sl],
                                                edt_ps[:, sl], 0.0)
                    nc.sync.dma_start(out=edt_row[0:1, :, sl],
                                      in_=edt_sb[:, sl])
                    nc.gpsimd.partition_broadcast(
                        ed_b[:, :, sl], edt_row[0:1, :, sl])

            # ---- slot loop ----
            with tc.tile_pool(name=f"agg_ps{layer}", bufs=1,
                              space="PSUM") as agg_pool:
                # k-major: chunk k of 128 dsts = one 2KB PSUM bank
                agg_ps = agg_pool.tile([128, NOWN, H, 128], F32, tag="agg",
                                       name=f"agg_{layer}")
                with tc.tile_pool(name=f"jl{layer}", bufs=4) as jl, \
                     tc.tile_pool(name=f"h_ps{layer}", bufs=1,
                                  space="PSUM") as h_psp:
                    for pos, s in enumerate(ORD[layer]):
                        lo, hi = WIN[s]
                        Ws = hi - lo
                        mn = mns[:, woff[s]:woff[s] + Ws]
                        if layer == 1:
                            h_ps = h_psp.tile([128, HC], F32, tag="h",
                                              name=f"h_ps_{s}")
                            nc.tensor.matmul(
                                h_ps[:, :],
                                sel5_sb[0:3, s * 128:(s + 1) * 128],
                                w1p_sb[:, :], start=True, stop=True)
                            nc.scalar.copy(
                                h65[:, s, :, 0:C],
                                h_ps[:, :].rearrange("p (h c) -> p h c",
                                                     h=H))
                            g_ps = h_psp.tile([128, GA + H], F32, tag="g",
                                              name=f"g_ps_{s}")
                            # one windowed g chunk + es ride columns
                            nc.tensor.matmul(
                                g_ps[:, 0:Ws],
                                sel5_sb[:, s * 128:(s + 1) * 128],
                                own5ge_sb[:, lo:hi],
                                start=True, stop=True)
                            nc.tensor.matmul(
                                g_ps[:, GA:GA + H],
                                sel5_sb[:, s * 128:(s + 1) * 128],
                                own5ge_sb[:, IC:IC + H],
                                start=True, stop=True)
                            nc.vector.tensor_scalar(
                                mn[:, 0:Ws], g_ps[:, 0:Ws], 0.0, MNEG,
                                OP.is_lt, OP.mult)
                            nc.vector.tensor_scalar_add(
                                es4f[:, s, :], g_ps[:, GA:GA + H], 0.0)
                            if s >= NOWN:
                                # index-only mask: remove the pairs the
                                # own slots already covered
                                nc.vector.tensor_tensor(
                                    mn[:, 0:Ws], mn[:, 0:Ws],
                                    wmn_sb[:, s - NOWN, lo:hi], OP.min)
                            src = hsrc[:, s, :]
                            es_ap = es4f[:, s, :]
                        else:
                            if s < NOWN:
                                # own-node slots: h2 rows resident in hg_sb
                                src = hg_sb[:, s, :]
                            else:
                                src = jl.tile([128, ROWW], F16, tag="hg",
                                              name=f"hg_{s}")
                                # fixup lanes from local hg rows (early,
                                # no AllGather dependency)...
                                nc.gpsimd.indirect_dma_start(
                                    out=src[:, :], out_offset=None,
                                    in_=hg_dram[:, :],
                                    in_offset=bass.IndirectOffsetOnAxis(
                                        ap=locidx_sb[:, s - NOWN:s - NOWN + 1],
                                        axis=0),
                                    bounds_check=IC - 1, oob_is_err=False)
                                # ...remote lanes from the AllGather
                                nc.gpsimd.indirect_dma_start(
                                    out=src[:, :], out_offset=None,
                                    in_=ag_out[:, :],
                                    in_offset=bass.IndirectOffsetOnAxis(
                                        ap=agidx_sb[:, s:s + 1], axis=0),
                                    bounds_check=n_cores * E - 1,
                                    oob_is_err=False)
                            esg = jl.tile([128, H], F32, tag="esg",
                                          name=f"esg_{s}")
                            nc.vector.tensor_scalar_add(
                                esg[:, :],
                                src[:, H * (C + 1):ROWW], 0.0)
                            es_ap = esg[:, :]

                        # scores: L = leaky02(ed + es + mn); A = exp(L).
                        # u4 = ed + mn in ONE 2x TT via a stride-0 head
                        # broadcast of mn.  Then heads 0-1 get es+leaky via
                        # ACT Prelu (bias=es); heads 2-3 via 4x TS es-adds
                        # and a TS/TT leaky (STT only has a 1x uop).
                        L4 = jl.tile([128, H, IC], F16, tag="L4",
                                     name=f"L4_{layer}_{s}")
                        u4 = jl.tile([128, H, IC], F16, tag="u4",
                                     name=f"u4_{layer}_{s}")
                        ub, mb = bass.broadcast_tensor_aps(
                            ed_b[:, :, lo:hi],
                            mn[:, 0:Ws].rearrange("p (o d) -> p o d", o=1))
                        nc.vector.tensor_tensor(u4[:, :, 0:Ws], ub, mb,
                                                OP.add)
                        for h in range(2):
                            nc.scalar.activation(
                                L4[:, h, 0:Ws], u4[:, h, 0:Ws], AF.Prelu,
                                bias=es_ap[:, h:h + 1],
                                scale=1.0, alpha=0.2)
                        v2 = jl.tile([128, 2, IC], F16, tag="v2",
                                     name=f"v2_{layer}_{s}")
                        for h in range(2, H):
                            nc.vector.tensor_scalar_add(
                                v2[:, h - 2, 0:Ws], u4[:, h, 0:Ws],
                                es_ap[:, h:h + 1])
                        t2 = jl.tile([128, 2, IC], F16, tag="t2",
                                     name=f"t2_{layer}_{s}")
                        nc.vector.tensor_scalar_mul(t2[:, :, 0:Ws],
                                                    v2[:, :, 0:Ws], 0.2)
                        nc.vector.tensor_tensor(L4[:, 2:4, 0:Ws],
                                                v2[:, :, 0:Ws],
                                                t2[:, :, 0:Ws], OP.max)
                        A4 = jl.tile([128, H, IC], F16, tag="A4",
                                     name=f"A4_{layer}_{s}")
                        nc.scalar.activation(A4[:, :, 0:Ws],
                                             L4[:, :, 0:Ws], AF.Exp)

                        # transposed aggregation: [h|ones] stationary,
                        # one 128-dst chunk (=one PSUM bank) per matmul.
                        # start=True (bank clear) only on the head-0
                        # matmul of the first slot touching chunk k.
                        for h in range(H):
                            for k in range(lo // 128, hi // 128):
                                nc.tensor.matmul(
                                    agg_ps[0:C + 1, k, h, :],
                                    src[:, h * (C + 1):(h + 1) * (C + 1)],
                                    A4[:, h, k * 128 - lo:k * 128 - lo + 128],
                                    start=(pos == first_pos[layer][k]
                                           and h == 0),
                                    stop=(pos == last_pos[layer][k]))

                # ---- finalize: x^T = relu(num*dinv + b) ----
                bt_sb = b1t_sb if layer == 1 else b2t_sb
                xT = x1T
                # k-group split: group A (chunks 0-2) stops accumulating
                # before group B in layer 1 (extras-first order), so its
                # whole den->dinv->broadcast->mult chain overlaps the
                # remaining own slots.
                KH = [slice(0, NOWN // 2), slice(NOWN // 2, NOWN)]
                fin_st = ExitStack()
                if layer == 2:
                    fc_ps_pool = fin_st.enter_context(
                        tc.tile_pool(name="fc", bufs=1, space="PSUM"))
                    logit_ps = fc_ps_pool.tile([128, IC // 128, 2], F32,
                                               tag="lg")
                # 1/den as exp(-ln(den)) straight from PSUM.  Strict
                # phase order Ln,Ln,Exp,Exp costs exactly two table
                # loads per layer (interleaving the groups would thrash
                # the ACT table back and forth).
                for g2 in range(2):
                    nc.scalar.activation(
                        den_sb[C:C + 1, KH[g2], :, :],
                        agg_ps[C:C + 1, KH[g2], :, :],
                        AF.Ln)
                for g2 in range(2):
                    nc.scalar.activation(
                        dinv_sb[C:C + 1, KH[g2], :, :],
                        den_sb[C:C + 1, KH[g2], :, :],
                        AF.Exp, scale=-1.0)
                for g2 in range(2):
                    ks = KH[g2]
                    nc.sync.dma_start(
                        out=dinv_row[0:1, ks, :, :],
                        in_=dinv_sb[C:C + 1, ks, :, :])
                    nc.gpsimd.partition_broadcast(
                        dinv_b[0:C, ks, :, :].rearrange(
                            "p k h d -> p (k h d)"),
                        dinv_row[0:1, ks, :, :].rearrange(
                            "p k h d -> p (k h d)"))
                    nc.vector.tensor_tensor(
                        xc[:, ks, :, :], agg_ps[0:C, ks, :, :],
                        dinv_b[0:C, ks, :, :], OP.mult)
                    gcol = slice(g2 * (IC // 2), (g2 + 1) * (IC // 2))
                    for h in range(H):
                        nc.vector.tensor_scalar(
                            xr[:, h, gcol].rearrange("p (k d) -> p k d",
                                                     k=NOWN // 2),
                            xc[:, ks, h, :], bt_sb[:, h:h + 1],
                            0.0, OP.add, OP.max)
                        if layer == 1:
                            po = (h % 2) * C
                            nc.sync.dma_start(
                                out=xT[po:po + C, h // 2, gcol],
                                in_=xr[0:C, h, gcol])
                        if layer == 2:
                            for oc in range(g2 * (NOWN // 2),
                                            (g2 + 1) * (NOWN // 2)):
                                nc.tensor.matmul(
                                    logit_ps[:, oc, :],
                                    xr[0:C, h, oc * 128:(oc + 1) * 128],
                                    fcw_sb[:, h, :],
                                    start=(g2 == 0 and h == 0
                                           and oc == 0),
                                    stop=(g2 == 1 and h == H - 1
                                          and oc == NOWN - 1))
                if layer == 2:
                    for o in range(2):
                        nc.vector.tensor_scalar_add(
                            logit_sb[:, :, o], logit_ps[:, :, o],
                            fcb_sb[:, o:o + 1])
                    nc.sync.dma_start(
                        out=out_d.rearrange("(q p) o -> p q o", p=128),
                        in_=logit_sb[:, :, :])
                fin_st.close()

            if layer == 1:
                # ---- h2 rows (+es) per own chunk; scatter exports and
                # AllGather the compact [E, ROWW] block.  Chunk-complete
                # order (s2 inner): chunks 0-2 only need x1T group A, so
                # their h2/export work overlaps the group-B finalize.
                with tc.tile_pool(name="h2", bufs=1, space="PSUM") as h2p:
                    h2_tiles = [h2p.tile([128, HCE], F32, tag=f"h2_{oc}",
                                         name=f"h2_{oc}")
                                for oc in range(IC // 128)]
                    for oc in range(IC // 128):
                        for s2 in range(2):
                            nc.tensor.matmul(
                                h2_tiles[oc][:, :],
                                x1T[:, s2, oc * 128:(oc + 1) * 128],
                                w2p_sb[:, s2, :],
                                start=(s2 == 0), stop=(s2 == 1))
                        if oc % 2 == 0:
                            nc.scalar.copy(
                                g65[:, oc, :, 0:C],
                                h2_tiles[oc][:, 0:HC].rearrange(
                                    "p (h c) -> p h c", h=H))
                        else:
                            nc.vector.tensor_scalar_add(
                                g65[:, oc, :, 0:C],
                                h2_tiles[oc][:, 0:HC].rearrange(
                                    "p (h c) -> p h c", h=H), 0.0)
                        nc.vector.tensor_scalar_add(
                            hg_sb[:, oc, H * (C + 1):ROWW],
                            h2_tiles[oc][:, HC:HCE], 0.0)
                        nc.sync.dma_start(
                            out=hg_dram.rearrange(
                                "(q p) r -> p q r", p=128)[:, oc, :],
                            in_=hg_sb[:, oc, :])
                # gather the E exported rows into the compact AG input
                # (2 gpsimd ops; the per-oc hg_dram DMAs above run on
                # the parallel DMA queues, keeping gpsimd free)
                for t in range(ET):
                    nc.gpsimd.indirect_dma_start(
                        out=exp_sb[:, t, :], out_offset=None,
                        in_=hg_dram[:, :],
                        in_offset=bass.IndirectOffsetOnAxis(
                            ap=expidx_sb[:, t:t + 1], axis=0))
                for t in range(ET):
                    r0 = t * 128
                    r1 = min(E, r0 + 128)
                    nc.sync.dma_start(out=hgx_dram[r0:r1, :],
                                      in_=exp_sb[0:r1 - r0, t, :])
                if fake_ag:
                    for r in range(n_cores):
                        nc.sync.dma_start(
                            out=ag_out[r * E:(r + 1) * E, :],
                            in_=hgx_dram[:, :])
                else:
                    nc.gpsimd.collective_compute(
                        "AllGather", OP.bypass,
                        replica_groups=[list(range(n_cores))],
                        ins=[hgx_dram.opt()],
                        outs=[ag_out.opt()])


    nc.compile()
    return nc


_BUILD_CACHE = {}


def _get_nc(nslot, nexp):
    key = (nslot, nexp)
    if key not in _BUILD_CACHE:
        _BUILD_CACHE[key] = build(nslot, nexp)
    return _BUILD_CACHE[key]


def _morton(p, bits=10):
    q = np.clip((p * (1 << bits)).astype(np.int64), 0, (1 << bits) - 1)
    code = np.zeros(len(p), np.int64)
    for b in range(bits):
        for dim in range(3):
            code |= ((q[:, dim] >> b) & 1) << (3 * b + dim)
    return code


def _plan(pts):
    """Sort nodes spatially; build per-core slot tiles.

    Own slots 0-5 are the core's identity Morton blocks restricted to
    their dst windows; "extra" tiles hold remote sources plus own fixup
    nodes (any out-of-window edge), processed over the full dst range
    with an index-only window mask to avoid double counting.
    """
    order = np.argsort(_morton(pts), kind="stable")
    p_sorted = np.full((KP, 3), PAD_COORD, np.float32)
    p_sorted[:K] = pts[order]

    sq = (p_sorted ** 2).sum(-1, dtype=np.float32)
    G = p_sorted @ p_sorted.T
    d2 = sq[None, :] + sq[:, None] - 2.0 * G
    near = d2 < (R2 + MASK_EPS)          # [src, dst], conservative superset

    win = _windows(NOWN)
    # per-core, per-384-segment source sets: a source belongs to segment
    # g if it has an "extra-responsibility" edge there (remote edge, or
    # own-fixup edge outside the node's own window)
    seg_sets = []
    for c in range(N_CORES):
        base = c * IC
        ncols = near[:, base:base + IC]
        srcs = np.flatnonzero(ncols.any(axis=1))
        per_seg = [set(), set()]
        for n in srcs:
            if base <= n < base + IC:
                lo, hi = win[(n - base) // 128]
                dsts = np.flatnonzero(ncols[n])
                dsts = dsts[(dsts < lo) | (dsts >= hi)]
            else:
                dsts = np.flatnonzero(ncols[n])
            for g in set(int(dd) // SEG for dd in dsts):
                per_seg[g].add(int(n))
        seg_sets.append([np.array(sorted(x), np.int64) for x in per_seg])
    tiles_per_seg = [max(-(-len(seg_sets[c][g]) // 128)
                         for c in range(N_CORES)) for g in range(2)]
    # extra tiles alternate segments A,B,A,B to match _windows
    assert tiles_per_seg[0] == tiles_per_seg[1], tiles_per_seg
    XT = tiles_per_seg[0] + tiles_per_seg[1]
    extras_list = []
    for c in range(N_CORES):
        tiles = []
        for i in range(XT):
            g = i % 2
            j = i // 2
            e = seg_sets[c][g][j * 128:(j + 1) * 128]
            tiles.append(np.concatenate(
                [e, np.full(128 - len(e), PAD_NODE, np.int64)]))
        extras_list.append(np.concatenate(tiles))
    T = NOWN + XT
    # export sets: rows of owner o consumed by any OTHER core
    exp_sets = [set() for _ in range(N_CORES)]
    for c in range(N_CORES):
        e = extras_list[c]
        rrem = e[(e != PAD_NODE) & ((e < c * IC) | (e >= (c + 1) * IC))]
        for r in rrem:
            exp_sets[int(r) // IC].add(int(r))
    exp_rows = [np.array(sorted(x), np.int64) for x in exp_sets]
    E = max(8, max(len(x) for x in exp_rows))
    return order, p_sorted, extras_list, T, exp_rows, E


def _blockdiag(a):  # [H, C] -> [HC, H] fp32
    m = np.zeros((HC, H), dtype=np.float32)
    for h in range(H):
        m[h * C:(h + 1) * C, h] = np.asarray(a, np.float32)[h]
    return m


def _prep_inputs(pos, pos_non_manifold, W1, a_src1, a_dst1, b1,
                 W2, a_src2, a_dst2, b2, fc_w, fc_b):
    f16 = np.float16
    pts = np.concatenate([np.asarray(pos, np.float32),
                          np.asarray(pos_non_manifold, np.float32)],
                         axis=2)[0].T  # [K, 3]
    order, p_sorted, extras_list, T, exp_rows, E = _plan(pts)
    XT = T - NOWN
    ET = -(-E // 128)
    win = _windows(NOWN)
    sq_sorted = (p_sorted ** 2).sum(-1, dtype=np.float32)
    # global node id -> AllGather row position (owner-block concat)
    ag_pos = np.full(KP, 0, np.int64)
    for o in range(N_CORES):
        ag_pos[exp_rows[o]] = o * E + np.arange(len(exp_rows[o]))

    W1f = np.asarray(W1, np.float32)
    W2f = np.asarray(W2, np.float32)
    w1s = W1f @ _blockdiag(a_src1)            # [3, H]
    w2p = np.concatenate([W2f, W2f @ _blockdiag(a_src2)], axis=1)

    shared = {
        "w1p": np.ascontiguousarray(W1f),
        "w1d": np.ascontiguousarray(W1f @ _blockdiag(a_dst1)),
        "w2p": np.ascontiguousarray(w2p.astype(f16)),
        "admw2": np.ascontiguousarray(
            (W2f @ _blockdiag(a_dst2)).astype(f16)),
        "b1t": np.ascontiguousarray(
            np.asarray(b1, np.float32).reshape(H, C).T),
        "b2t": np.ascontiguousarray(
            np.asarray(b2, np.float32).reshape(H, C).T),
        "fcw": np.ascontiguousarray(np.asarray(fc_w, np.float32).reshape(
            H, C, 2).transpose(1, 0, 2).astype(f16)),
        "fcb": np.ascontiguousarray(np.broadcast_to(
            np.asarray(fc_b, np.float32).reshape(1, 2), (128, 2))),
    }
    in_maps = []
    for c in range(N_CORES):
        base = c * IC
        own = np.arange(base, base + IC, dtype=np.int64)
        extras = extras_list[c]
        srcs = np.concatenate([own, extras])          # [T*128]
        psel = p_sorted[srcs]                         # [T*128, 3]
        pown = p_sorted[base:base + IC]
        sel5 = np.concatenate(
            [psel.T, sq_sorted[srcs][None, :],
             np.ones((1, len(srcs)), np.float32)], axis=0)
        own5 = np.concatenate(
            [2.0 * pown.T, -np.ones((1, IC), np.float32),
             (R2 - sq_sorted[base:base + IC])[None, :]], axis=0)
        es_cols = np.concatenate(
            [w1s, np.zeros((2, H), np.float32)], axis=0)  # [5, H]
        # index-only window mask for extra-slot lanes: for own fixup
        # lanes, kill the dsts the own slot already covered.
        wmn = np.zeros((XT, 128, IC), np.float16)
        locidx = np.full((XT, 128), BIGIDX, np.int64)
        for t in range(XT):
            for p in range(128):
                n = int(extras[t * 128 + p])
                if n == PAD_NODE:
                    locidx[t, p] = 0          # safe local row, fully masked
                elif base <= n < base + IC:
                    locidx[t, p] = n - base   # own fixup lane
                    lo, hi = win[(n - base) // 128]
                    wmn[t, p, lo:hi] = MNEG
        agidx = ag_pos[srcs].copy()
        isrem = (srcs != PAD_NODE) & ((srcs < base) | (srcs >= base + IC))
        agidx[~isrem] = BIGIDX                # skip non-remote lanes
        m = dict(shared)
        m["sel5"] = np.ascontiguousarray(sel5)
        m["own5ge"] = np.ascontiguousarray(
            np.concatenate([own5, es_cols], axis=1))
        m["own3"] = np.ascontiguousarray(pown.T)
        m["agidx"] = np.ascontiguousarray(
            agidx.reshape(T, 128).T.astype(np.int32))
        m["locidx"] = np.ascontiguousarray(locidx.T.astype(np.int32))
        m["wmn"] = np.ascontiguousarray(wmn.transpose(1, 0, 2))
        eloc = np.concatenate(
            [exp_rows[c] - base,
             np.zeros(ET * 128 - len(exp_rows[c]), np.int64)])
        m["expidx"] = np.ascontiguousarray(
            eloc.reshape(ET, 128).T.astype(np.int32))
        in_maps.append(m)
    return in_maps, order, T, E


def kernel(pos, pos_non_manifold, W1, a_src1, a_dst1, b1,
           W2, a_src2, a_dst2, b2, fc_w, fc_b, _trace=False):
    in_maps, order, T, E = _prep_inputs(
        pos, pos_non_manifold, W1, a_src1, a_dst1, b1,
        W2, a_src2, a_dst2, b2, fc_w, fc_b)
    nc = _get_nc(T, E)
    res = run_bass_kernel_spmd(nc, in_maps, core_ids=list(range(N_CORES)),
                               trace=_trace)
    kernel.last_results = res
    x2s = np.concatenate([res.results[c]["out"] for c in range(N_CORES)],
                         axis=0)  # [KP, 2] in sorted order
    x2 = np.empty((K, 2), np.float32)
    x2[order] = x2s[:K]
    logits = np.ascontiguousarray(x2[M:K]).reshape(1, 2, 3000)
    return logits.astype(np.float32)
